# revision 4
# baseline (speedup 1.0000x reference)
"""Trainium2 Bass kernel for causal multi-head attention with LoRA (QKV + proj).

Problem (hardcoded): B=4, T=2048, C=1024, NH=16, HD=64, RANK=56, alpha=8.

Sharding: tensor-parallel across heads — each of the 8 cores owns 2 heads
(128 qkv dims per projection) and processes all 4 batches. The output
projection is row-parallel (each core contracts over its own 128 y dims);
partial outputs are summed on the host.

All matmuls run in float32r (TF32-like, ~1e-4 relative rounding, full PE
rate for moving dims >= 256). Layout is transposed throughout: activations
live as [feature(partition), token(free)], which makes the QKV projection,
QK^T, PV and output projection all natural matmuls. The only on-chip
transposes are V (needed token-major for PV): 16 PE transposes per batch.

Softmax: scoresT [tk, tq] -> exp on ACT (scale=1/8 folded in); causal
masking via block-sliced matmul ranges + one [128,128] triangular mask
multiply per diagonal block; row sums via an appended ones column in the
PV stationary operand; normalization via reciprocal + K=1 broadcast matmul.
"""
import sys
import numpy as np

if "/opt/trn_rl_repo" not in sys.path:
    sys.path.insert(0, "/opt/trn_rl_repo")

import concourse.bass as bass  # noqa: E402
from concourse import bacc  # noqa: E402
import concourse.mybir as mybir  # noqa: E402
import concourse.tile as tile  # noqa: E402
from concourse.bass_utils import run_bass_kernel_spmd  # noqa: E402

B, T, C = 4, 2048, 1024
NH, HD, RANK = 16, 64, 56
SCALING = 8.0 / 56.0
NCORES = 8
BT = B * T            # 8192
TOK = 512             # token chunk (matmul moving dim)
NT4 = T // TOK        # 4 token chunks per batch
NCIN = C // 128       # 8 input-feature chunks
NCO = C // 128        # 8 output-feature chunks (proj)
F32 = mybir.dt.float32
F32R = mybir.dt.float32r
EXPF = mybir.ActivationFunctionType.Exp
COPYF = mybir.ActivationFunctionType.Identity  # Copy rejects AP bias

_cache = {}


def _build():
    nc = bacc.Bacc("TRN2", target_bir_lowering=False, debug=False,
                   num_devices=NCORES)
    xT = nc.dram_tensor("xT", [C, BT], F32R, kind="ExternalInput")
    Wq = nc.dram_tensor("Wq", [NCIN, 128, 384], F32R, kind="ExternalInput")
    bq = nc.dram_tensor("bq", [128, 3], F32, kind="ExternalInput")
    Aq = nc.dram_tensor("Aq", [NCIN, 128, RANK], F32R, kind="ExternalInput")
    Bq = nc.dram_tensor("Bq", [RANK, 384], F32R, kind="ExternalInput")
    Wp = nc.dram_tensor("Wp", [128, C], F32R, kind="ExternalInput")
    Ap = nc.dram_tensor("Ap", [128, RANK], F32R, kind="ExternalInput")
    Bp = nc.dram_tensor("Bp", [RANK, C], F32R, kind="ExternalInput")
    tri = nc.dram_tensor("tri", [128, 128], F32R, kind="ExternalInput")
    ones64 = nc.dram_tensor("ones64", [1, 64], F32R, kind="ExternalInput")
    eye = nc.dram_tensor("eye", [128, 128], F32R, kind="ExternalInput")
    onesb = nc.dram_tensor("onesb", [128, 16], F32R, kind="ExternalInput")
    outT = nc.dram_tensor("outT", [C, BT], F32, kind="ExternalOutput")

    with tile.TileContext(nc) as tc:
        with (
            tc.tile_pool(name="consts", bufs=1) as consts,
            tc.tile_pool(name="qkv", bufs=2) as qkvp,
            tc.tile_pool(name="vaugp", bufs=2) as vaugp,
            tc.tile_pool(name="ytp", bufs=2) as ytp,
            tc.tile_pool(name="xtp", bufs=16) as xtp,
            tc.tile_pool(name="expp", bufs=6) as expp,
            tc.tile_pool(name="small", bufs=2) as small,
            tc.tile_pool(name="ps", bufs=1, space="PSUM") as ps,
        ):
            wq_sb = consts.tile([128, NCIN, 384], F32R)
            nc.sync.dma_start(wq_sb[:], Wq[:].rearrange("c p f -> p c f"))
            aq_sb = consts.tile([128, NCIN, RANK], F32R)
            nc.sync.dma_start(aq_sb[:], Aq[:].rearrange("c p f -> p c f"))
            lb_sb = consts.tile([RANK, 384], F32R)
            nc.sync.dma_start(lb_sb[:], Bq[:])
            bias_sb = consts.tile([128, 3], F32)
            nc.sync.dma_start(bias_sb[:], bq[:])
            wp_sb = consts.tile([128, C], F32R)
            nc.sync.dma_start(wp_sb[:], Wp[:])
            ap_sb = consts.tile([128, RANK], F32R)
            nc.sync.dma_start(ap_sb[:], Ap[:])
            pb_sb = consts.tile([RANK, C], F32R)
            nc.sync.dma_start(pb_sb[:], Bp[:])
            tri_sb = consts.tile([128, 128], F32R)
            nc.sync.dma_start(tri_sb[:], tri[:])
            ones_sb = consts.tile([1, 64], F32R)
            nc.sync.dma_start(ones_sb[:], ones64[:])
            eye_sb = consts.tile([128, 128], F32R)
            nc.sync.dma_start(eye_sb[:], eye[:])

            for b in range(B):
                # ---- QKV projection (+ LoRA) for batch b ----
                qT = qkvp.tile([128, T], F32R, tag="qT")
                kT = qkvp.tile([128, T], F32R, tag="kT")
                vT = qkvp.tile([128, T], F32R, tag="vT")
                with nc.named_scope(f"qkv{b}"):
                    for t4 in range(NT4):
                        gcol = b * T + t4 * TOK
                        xts = []
                        for cc in range(NCIN):
                            xt = xtp.tile([128, TOK], F32R, tag="xt")
                            nc.sync.dma_start(
                                xt[:],
                                xT[cc * 128:(cc + 1) * 128, gcol:gcol + TOK])
                            xts.append(xt)
                        ps_r = ps.tile([RANK, TOK], F32, tag="accr", bufs=1)
                        for cc in range(NCIN):
                            nc.tensor.matmul(ps_r[:], aq_sb[:, cc, :], xts[cc][:],
                                             start=(cc == 0), stop=(cc == NCIN - 1))
                        r_sb = small.tile([RANK, TOK], F32R, tag="r")
                        nc.vector.tensor_copy(r_sb[:], ps_r[:])
                        for ch, dest in enumerate((qT, kT, vT)):
                            ps_q = ps.tile([128, TOK], F32, tag="acc", bufs=2)
                            for cc in range(NCIN):
                                nc.tensor.matmul(
                                    ps_q[:],
                                    wq_sb[:, cc, ch * 128:(ch + 1) * 128],
                                    xts[cc][:], start=(cc == 0), stop=False)
                            nc.tensor.matmul(
                                ps_q[:], lb_sb[:, ch * 128:(ch + 1) * 128],
                                r_sb[:], start=False, stop=True)
                            nc.scalar.activation(
                                out=dest[:, t4 * TOK:(t4 + 1) * TOK],
                                in_=ps_q[:], func=COPYF,
                                bias=bias_sb[:, ch:ch + 1], scale=1.0)

                # ---- V -> token-major (+ ones cols) ----
                vaug = vaugp.tile([128, 16, 130], F32R, tag="vaug")
                with nc.named_scope(f"vtr{b}"):
                    nc.sync.dma_start(vaug[:, :, 64:65],
                                      onesb[:].unsqueeze(-1))
                    nc.sync.dma_start(vaug[:, :, 129:130],
                                      onesb[:].unsqueeze(-1))
                    for tb in range(16):
                        ps_t = ps.tile([128, 128], F32R, tag="mm", bufs=3)
                        nc.tensor.transpose(ps_t[:], vT[:, tb * 128:(tb + 1) * 128],
                                            eye_sb[:])
                        nc.vector.tensor_copy(vaug[:, tb, 0:64], ps_t[:, 0:64])
                        nc.vector.tensor_copy(vaug[:, tb, 65:129], ps_t[:, 64:128])

                # ---- attention ----
                yt = ytp.tile([128, T], F32R, tag="yt")
                with nc.named_scope(f"attn{b}"):
                    for h in range(2):
                        hp = h * 64
                        for t4 in range(NT4):
                            nblk = 4 * (t4 + 1)
                            q0s, exps = {}, {}
                            ps_y = ps.tile([65, TOK], F32, tag="accy", bufs=2)

                            def emit_qk(j, t4=t4, h=h, hp=hp, q0s=q0s, exps=exps):
                                r = j - 4 * t4
                                q0 = 128 * r if r > 0 else 0
                                q0s[j] = q0
                                ps_s = ps.tile([128, TOK], F32, tag="mm", bufs=3)
                                nc.tensor.matmul(
                                    ps_s[:, q0:TOK],
                                    kT[hp:hp + 64, j * 128:(j + 1) * 128],
                                    qT[hp:hp + 64, t4 * TOK + q0:(t4 + 1) * TOK],
                                    start=True, stop=True)
                                e = expp.tile([128, TOK], F32R, tag="expS")
                                nc.scalar.activation(
                                    out=e[:, q0:TOK], in_=ps_s[:, q0:TOK],
                                    func=EXPF, scale=0.125)
                                if r >= 0:
                                    nc.vector.tensor_mul(
                                        e[:, q0:q0 + 128], e[:, q0:q0 + 128],
                                        tri_sb[:])
                                exps[j] = e

                            def emit_pv(j, h=h, nblk=nblk, q0s=q0s, exps=exps,
                                        ps_y=ps_y, vaug=vaug):
                                q0 = q0s[j]
                                nc.tensor.matmul(
                                    ps_y[:, q0:TOK],
                                    vaug[:, j, 65 * h:65 * h + 65],
                                    exps[j][:, q0:TOK],
                                    start=(j == 0), stop=(j == nblk - 1))

                            emit_qk(0)
                            if nblk > 1:
                                emit_qk(1)
                            for j in range(nblk):
                                if j + 2 < nblk:
                                    emit_qk(j + 2)
                                emit_pv(j)

                            recip = small.tile([65, TOK], F32R, tag="recip")
                            with nc.allow_low_precision(reason="f32r recip"):
                                nc.vector.reciprocal(recip[64:65, :],
                                                     ps_y[64:65, :])
                            recip0 = small.tile([1, TOK], F32R, tag="recip0")
                            nc.sync.dma_start(recip0[:], recip[64:65, :])
                            ps_b = ps.tile([64, TOK], F32, tag="mm", bufs=3)
                            nc.tensor.matmul(ps_b[:], ones_sb[:], recip0[:],
                                             start=True, stop=True)
                            sb_b = small.tile([64, TOK], F32R, tag="sbb")
                            nc.vector.tensor_copy(sb_b[:], ps_b[:])
                            tsl = slice(t4 * TOK, (t4 + 1) * TOK)
                            if h == 0:
                                nc.vector.tensor_mul(yt[0:64, tsl],
                                                     ps_y[0:64, :], sb_b[:])
                            else:
                                stage = small.tile([64, TOK], F32R, tag="stage")
                                nc.vector.tensor_mul(stage[:], ps_y[0:64, :],
                                                     sb_b[:])
                                nc.sync.dma_start(yt[64:128, tsl], stage[:])

                # ---- output projection (+ LoRA), row-parallel partial ----
                with nc.named_scope(f"proj{b}"):
                    for t4 in range(NT4):
                        gcol = b * T + t4 * TOK
                        tsl = slice(t4 * TOK, (t4 + 1) * TOK)
                        ps_rp = ps.tile([RANK, TOK], F32, tag="accr", bufs=1)
                        nc.tensor.matmul(ps_rp[:], ap_sb[:], yt[:, tsl],
                                         start=True, stop=True)
                        rp_sb = small.tile([RANK, TOK], F32R, tag="rp")
                        nc.vector.tensor_copy(rp_sb[:], ps_rp[:])
                        for co in range(NCO):
                            ps_o = ps.tile([128, TOK], F32, tag="acc", bufs=2)
                            nc.tensor.matmul(
                                ps_o[:], wp_sb[:, co * 128:(co + 1) * 128],
                                yt[:, tsl], start=True, stop=False)
                            nc.tensor.matmul(
                                ps_o[:], pb_sb[:, co * 128:(co + 1) * 128],
                                rp_sb[:], start=False, stop=True)
                            po = small.tile([128, TOK], F32, tag="po", bufs=3)
                            nc.vector.tensor_copy(po[:], ps_o[:])
                            nc.sync.dma_start(
                                outT[co * 128:(co + 1) * 128, gcol:gcol + TOK],
                                po[:])
    nc.compile()
    return nc


def _prep_inputs(x, W_attn, b_attn, A_attn, B_attn, W_proj, b_proj, A_proj,
                 B_proj):
    xT = np.ascontiguousarray(x.reshape(BT, C).T)
    AqT = np.ascontiguousarray(A_attn.T).reshape(NCIN, 128, RANK)
    ApT_full = A_proj  # [RANK, C]
    tri = np.triu(np.ones((128, 128), np.float32))
    ones64 = np.ones((1, 64), np.float32)
    eye = np.eye(128, dtype=np.float32)
    Bp_s = np.ascontiguousarray((B_proj * SCALING).T)  # [RANK, C]
    in_maps = []
    for c in range(NCORES):
        rows = np.r_[128 * c:128 * c + 128,
                     C + 128 * c:C + 128 * c + 128,
                     2 * C + 128 * c:2 * C + 128 * c + 128]
        W_sl = W_attn[rows]                                  # [384, C]
        WqT = np.ascontiguousarray(W_sl.T).reshape(NCIN, 128, 384)
        b_sl = np.ascontiguousarray(b_attn[rows].reshape(3, 128).T)
        Bq_s = np.ascontiguousarray((B_attn[rows] * SCALING).T)  # [RANK, 384]
        ysl = slice(128 * c, 128 * c + 128)
        WpT = np.ascontiguousarray(W_proj[:, ysl].T)         # [128, C]
        ApT = np.ascontiguousarray(ApT_full[:, ysl].T)       # [128, RANK]
        in_maps.append({
            "xT": xT, "Wq": WqT, "bq": b_sl, "Aq": AqT, "Bq": Bq_s,
            "Wp": WpT, "Ap": ApT, "Bp": Bp_s, "tri": tri,
            "ones64": ones64, "eye": eye, "onesb": np.ones((128, 16), np.float32),
        })
    return in_maps


def _install_ntff_shim():
    """Provide antenv.axon_hooks (missing on this image) via ctypes against
    the axon .so, mirroring trn_agent_boot.trn_boot._ntff_profile_via_ctypes."""
    import types
    import ctypes
    import contextlib
    try:
        from antenv.axon_hooks import get_axon_ntff_profile_hook  # noqa: F401
        return
    except ImportError:
        pass
    so_path = "/opt/axon/libaxon_pjrt.so"
    try:
        lib = ctypes.CDLL(so_path)
    except OSError:
        return
    if not hasattr(lib, "axon_start_nrt_profile"):
        return
    lib.axon_start_nrt_profile.argtypes = [ctypes.POINTER(ctypes.c_int64),
                                           ctypes.c_size_t]
    lib.axon_start_nrt_profile.restype = ctypes.c_int64
    lib.axon_stop_nrt_profile.argtypes = [ctypes.c_char_p]
    lib.axon_stop_nrt_profile.restype = ctypes.c_int64

    @contextlib.contextmanager
    def _hook(output_dir, device_ids):
        import jax
        jax.devices()
        if device_ids:
            ids = (ctypes.c_int64 * len(device_ids))(*device_ids)
            rc = lib.axon_start_nrt_profile(ids, len(device_ids))
        else:
            rc = lib.axon_start_nrt_profile(None, 0)
        if rc != 0:
            raise RuntimeError(f"axon_start_nrt_profile rc={rc}")
        try:
            yield
        finally:
            n = lib.axon_stop_nrt_profile(str(output_dir).encode())
            if n < 0:
                raise RuntimeError(f"axon_stop_nrt_profile rc={n}")

    import antenv
    mod = types.ModuleType("antenv.axon_hooks")
    mod.get_axon_ntff_profile_hook = lambda: _hook
    mod.set_axon_ntff_profile_hook = lambda h: None
    sys.modules["antenv.axon_hooks"] = mod
    antenv.axon_hooks = mod


def run(inputs, trace=False, trace_cores=None):
    """Run the kernel. Returns (output, BassKernelResults)."""
    if "nc" not in _cache:
        _cache["nc"] = _build()
    nc = _cache["nc"]
    inputs = {k: np.asarray(v, dtype=np.float32) for k, v in inputs.items()}
    in_maps = _prep_inputs(**inputs)
    if trace:
        _install_ntff_shim()
    res = run_bass_kernel_spmd(nc, in_maps, core_ids=list(range(NCORES)),
                               trace=trace, trace_cores=trace_cores)
    outT = np.zeros((C, BT), np.float64)
    for r in res.results:
        outT += r["outT"].astype(np.float64)
    out = outT.T + inputs["b_proj"][None, :]
    return out.astype(np.float32).reshape(B, T, C), res


def kernel(**inputs):
    out, _ = run(inputs, trace=False)
    return out


# revision 5
# speedup vs baseline: 1.1538x; 1.1538x over previous
"""Trainium2 Bass kernel for causal multi-head attention with LoRA (QKV + proj).

Problem (hardcoded): B=4, T=2048, C=1024, NH=16, HD=64, RANK=56, alpha=8.

Sharding: tensor-parallel across heads — each of the 8 cores owns 2 heads
(128 qkv dims per projection) and processes all 4 batches. The output
projection is row-parallel (each core contracts over its own 128 y dims);
partial outputs are summed on the host.

All matmuls run in float32r (TF32-like, ~1e-4 relative rounding, full PE
rate for moving dims >= 256). Layout is transposed throughout: activations
live as [feature(partition), token(free)], which makes the QKV projection,
QK^T, PV and output projection all natural matmuls. The only on-chip
transposes are V (needed token-major for PV): 16 PE transposes per batch.

Softmax: scoresT [tk, tq] -> exp on ACT (scale=1/8 folded in); causal
masking via block-sliced matmul ranges + one [128,128] triangular mask
multiply per diagonal block; row sums via an appended ones column in the
PV stationary operand; normalization via reciprocal + K=1 broadcast matmul.
"""
import sys
import numpy as np

if "/opt/trn_rl_repo" not in sys.path:
    sys.path.insert(0, "/opt/trn_rl_repo")

import concourse.bass as bass  # noqa: E402
from concourse import bacc  # noqa: E402
import concourse.mybir as mybir  # noqa: E402
import concourse.tile as tile  # noqa: E402
from concourse.bass_utils import run_bass_kernel_spmd  # noqa: E402

B, T, C = 4, 2048, 1024
NH, HD, RANK = 16, 64, 56
SCALING = 8.0 / 56.0
NCORES = 8
BT = B * T            # 8192
TOK = 512             # token chunk (matmul moving dim)
NT4 = T // TOK        # 4 token chunks per batch
NCIN = C // 128       # 8 input-feature chunks
NCO = C // 128        # 8 output-feature chunks (proj)
F32 = mybir.dt.float32
F32R = mybir.dt.float32r
F16 = mybir.dt.float16
EXPF = mybir.ActivationFunctionType.Exp
COPYF = mybir.ActivationFunctionType.Identity  # Copy rejects AP bias

_cache = {}


def _build():
    nc = bacc.Bacc("TRN2", target_bir_lowering=False, debug=False,
                   num_devices=NCORES)
    xT = nc.dram_tensor("xT", [C, BT], F16, kind="ExternalInput")
    Wq = nc.dram_tensor("Wq", [NCIN, 128, 384], F16, kind="ExternalInput")
    bq = nc.dram_tensor("bq", [128, 3], F32, kind="ExternalInput")
    Aq = nc.dram_tensor("Aq", [NCIN, 128, RANK], F16, kind="ExternalInput")
    Bq = nc.dram_tensor("Bq", [RANK, 384], F16, kind="ExternalInput")
    Wp = nc.dram_tensor("Wp", [128, C], F16, kind="ExternalInput")
    Ap = nc.dram_tensor("Ap", [128, RANK], F16, kind="ExternalInput")
    Bp = nc.dram_tensor("Bp", [RANK, C], F16, kind="ExternalInput")
    tri = nc.dram_tensor("tri", [128, 128], F16, kind="ExternalInput")
    ones64 = nc.dram_tensor("ones64", [1, 64], F16, kind="ExternalInput")
    eye = nc.dram_tensor("eye", [128, 128], F16, kind="ExternalInput")
    onesb = nc.dram_tensor("onesb", [128, 16], F16, kind="ExternalInput")
    outT = nc.dram_tensor("outT", [C, BT], F32, kind="ExternalOutput")

    with tile.TileContext(nc) as tc:
        with (
            tc.tile_pool(name="consts", bufs=1) as consts,
            tc.tile_pool(name="qkv", bufs=2) as qkvp,
            tc.tile_pool(name="vaugp", bufs=2) as vaugp,
            tc.tile_pool(name="ytp", bufs=2) as ytp,
            tc.tile_pool(name="xtp", bufs=16) as xtp,
            tc.tile_pool(name="expp", bufs=6) as expp,
            tc.tile_pool(name="small", bufs=2) as small,
            tc.tile_pool(name="ps", bufs=1, space="PSUM") as ps,
        ):
            wq_sb = consts.tile([128, NCIN, 384], F16)
            nc.sync.dma_start(wq_sb[:], Wq[:].rearrange("c p f -> p c f"))
            aq_sb = consts.tile([128, NCIN, RANK], F16)
            nc.sync.dma_start(aq_sb[:], Aq[:].rearrange("c p f -> p c f"))
            lb_sb = consts.tile([RANK, 384], F16)
            nc.sync.dma_start(lb_sb[:], Bq[:])
            bias_sb = consts.tile([128, 3], F32)
            nc.sync.dma_start(bias_sb[:], bq[:])
            wp_sb = consts.tile([128, C], F16)
            nc.sync.dma_start(wp_sb[:], Wp[:])
            ap_sb = consts.tile([128, RANK], F16)
            nc.sync.dma_start(ap_sb[:], Ap[:])
            pb_sb = consts.tile([RANK, C], F16)
            nc.sync.dma_start(pb_sb[:], Bp[:])
            tri_sb = consts.tile([128, 128], F16)
            nc.sync.dma_start(tri_sb[:], tri[:])
            ones_sb = consts.tile([1, 64], F16)
            nc.sync.dma_start(ones_sb[:], ones64[:])
            eye_sb = consts.tile([128, 128], F16)
            nc.sync.dma_start(eye_sb[:], eye[:])

            for b in range(B):
                # ---- QKV projection (+ LoRA) for batch b ----
                qT = qkvp.tile([128, T], F16, tag="qT")
                kT = qkvp.tile([128, T], F16, tag="kT")
                vT = qkvp.tile([128, T], F16, tag="vT")
                with nc.named_scope(f"qkv{b}"):
                    for t4 in range(NT4):
                        gcol = b * T + t4 * TOK
                        xts = []
                        for cc in range(NCIN):
                            xt = xtp.tile([128, TOK], F16, tag="xt")
                            nc.sync.dma_start(
                                xt[:],
                                xT[cc * 128:(cc + 1) * 128, gcol:gcol + TOK])
                            xts.append(xt)
                        ps_r = ps.tile([RANK, TOK], F32, tag="accr", bufs=1)
                        for cc in range(NCIN):
                            nc.tensor.matmul(ps_r[:], aq_sb[:, cc, :], xts[cc][:],
                                             start=(cc == 0), stop=(cc == NCIN - 1))
                        r_sb = small.tile([RANK, TOK], F16, tag="r")
                        nc.vector.tensor_copy(r_sb[:], ps_r[:])
                        for ch, dest in enumerate((qT, kT, vT)):
                            ps_q = ps.tile([128, TOK], F32, tag="acc", bufs=2)
                            for cc in range(NCIN):
                                nc.tensor.matmul(
                                    ps_q[:],
                                    wq_sb[:, cc, ch * 128:(ch + 1) * 128],
                                    xts[cc][:], start=(cc == 0), stop=False)
                            nc.tensor.matmul(
                                ps_q[:], lb_sb[:, ch * 128:(ch + 1) * 128],
                                r_sb[:], start=False, stop=True)
                            nc.scalar.activation(
                                out=dest[:, t4 * TOK:(t4 + 1) * TOK],
                                in_=ps_q[:], func=COPYF,
                                bias=bias_sb[:, ch:ch + 1], scale=1.0)

                # ---- V -> token-major (+ ones cols) ----
                vaug = vaugp.tile([128, 16, 130], F16, tag="vaug")
                with nc.named_scope(f"vtr{b}"):
                    nc.sync.dma_start(vaug[:, :, 64:65],
                                      onesb[:].unsqueeze(-1))
                    nc.sync.dma_start(vaug[:, :, 129:130],
                                      onesb[:].unsqueeze(-1))
                    for tb in range(16):
                        ps_t = ps.tile([128, 128], F16, tag="mm", bufs=3)
                        nc.tensor.transpose(ps_t[:], vT[:, tb * 128:(tb + 1) * 128],
                                            eye_sb[:])
                        nc.vector.tensor_copy(vaug[:, tb, 0:64], ps_t[:, 0:64])
                        nc.vector.tensor_copy(vaug[:, tb, 65:129], ps_t[:, 64:128])

                # ---- attention ----
                yt = ytp.tile([128, T], F16, tag="yt")
                with nc.named_scope(f"attn{b}"):
                    for h in range(2):
                        hp = h * 64
                        for t4 in range(NT4):
                            nblk = 4 * (t4 + 1)
                            q0s, exps = {}, {}
                            ps_y = ps.tile([65, TOK], F32, tag="accy", bufs=2)

                            def emit_qk(j, t4=t4, h=h, hp=hp, q0s=q0s, exps=exps):
                                r = j - 4 * t4
                                q0 = 128 * r if r > 0 else 0
                                q0s[j] = q0
                                ps_s = ps.tile([128, TOK], F32, tag="mm", bufs=3)
                                nc.tensor.matmul(
                                    ps_s[:, q0:TOK],
                                    kT[hp:hp + 64, j * 128:(j + 1) * 128],
                                    qT[hp:hp + 64, t4 * TOK + q0:(t4 + 1) * TOK],
                                    start=True, stop=True)
                                e = expp.tile([128, TOK], F16, tag="expS")
                                nc.scalar.activation(
                                    out=e[:, q0:TOK], in_=ps_s[:, q0:TOK],
                                    func=EXPF, scale=0.125)
                                if r >= 0:
                                    nc.vector.tensor_mul(
                                        e[:, q0:q0 + 128], e[:, q0:q0 + 128],
                                        tri_sb[:])
                                exps[j] = e

                            def emit_pv(j, h=h, nblk=nblk, q0s=q0s, exps=exps,
                                        ps_y=ps_y, vaug=vaug):
                                q0 = q0s[j]
                                nc.tensor.matmul(
                                    ps_y[:, q0:TOK],
                                    vaug[:, j, 65 * h:65 * h + 65],
                                    exps[j][:, q0:TOK],
                                    start=(j == 0), stop=(j == nblk - 1))

                            emit_qk(0)
                            if nblk > 1:
                                emit_qk(1)
                            for j in range(nblk):
                                if j + 2 < nblk:
                                    emit_qk(j + 2)
                                emit_pv(j)

                            recip = small.tile([65, TOK], F16, tag="recip")
                            with nc.allow_low_precision(reason="f32r recip"):
                                nc.vector.reciprocal(recip[64:65, :],
                                                     ps_y[64:65, :])
                            recip0 = small.tile([1, TOK], F16, tag="recip0")
                            nc.sync.dma_start(recip0[:], recip[64:65, :])
                            ps_b = ps.tile([64, TOK], F32, tag="mm", bufs=3)
                            nc.tensor.matmul(ps_b[:], ones_sb[:], recip0[:],
                                             start=True, stop=True)
                            sb_b = small.tile([64, TOK], F16, tag="sbb")
                            nc.vector.tensor_copy(sb_b[:], ps_b[:])
                            tsl = slice(t4 * TOK, (t4 + 1) * TOK)
                            if h == 0:
                                nc.vector.tensor_mul(yt[0:64, tsl],
                                                     ps_y[0:64, :], sb_b[:])
                            else:
                                stage = small.tile([64, TOK], F16, tag="stage")
                                nc.vector.tensor_mul(stage[:], ps_y[0:64, :],
                                                     sb_b[:])
                                nc.sync.dma_start(yt[64:128, tsl], stage[:])

                # ---- output projection (+ LoRA), row-parallel partial ----
                with nc.named_scope(f"proj{b}"):
                    for t4 in range(NT4):
                        gcol = b * T + t4 * TOK
                        tsl = slice(t4 * TOK, (t4 + 1) * TOK)
                        ps_rp = ps.tile([RANK, TOK], F32, tag="accr", bufs=1)
                        nc.tensor.matmul(ps_rp[:], ap_sb[:], yt[:, tsl],
                                         start=True, stop=True)
                        rp_sb = small.tile([RANK, TOK], F16, tag="rp")
                        nc.vector.tensor_copy(rp_sb[:], ps_rp[:])
                        for co in range(NCO):
                            ps_o = ps.tile([128, TOK], F32, tag="acc", bufs=2)
                            nc.tensor.matmul(
                                ps_o[:], wp_sb[:, co * 128:(co + 1) * 128],
                                yt[:, tsl], start=True, stop=False)
                            nc.tensor.matmul(
                                ps_o[:], pb_sb[:, co * 128:(co + 1) * 128],
                                rp_sb[:], start=False, stop=True)
                            po = small.tile([128, TOK], F32, tag="po", bufs=3)
                            nc.vector.tensor_copy(po[:], ps_o[:])
                            nc.sync.dma_start(
                                outT[co * 128:(co + 1) * 128, gcol:gcol + TOK],
                                po[:])
    nc.compile()
    return nc


def _prep_inputs(x, W_attn, b_attn, A_attn, B_attn, W_proj, b_proj, A_proj,
                 B_proj):
    xT = np.ascontiguousarray(x.reshape(BT, C).T)
    AqT = np.ascontiguousarray(A_attn.T).reshape(NCIN, 128, RANK)
    ApT_full = A_proj  # [RANK, C]
    tri = np.triu(np.ones((128, 128), np.float32))
    ones64 = np.ones((1, 64), np.float32)
    eye = np.eye(128, dtype=np.float32)
    Bp_s = np.ascontiguousarray((B_proj * SCALING).T)  # [RANK, C]
    in_maps = []
    for c in range(NCORES):
        rows = np.r_[128 * c:128 * c + 128,
                     C + 128 * c:C + 128 * c + 128,
                     2 * C + 128 * c:2 * C + 128 * c + 128]
        W_sl = W_attn[rows]                                  # [384, C]
        WqT = np.ascontiguousarray(W_sl.T).reshape(NCIN, 128, 384)
        b_sl = np.ascontiguousarray(b_attn[rows].reshape(3, 128).T)
        Bq_s = np.ascontiguousarray((B_attn[rows] * SCALING).T)  # [RANK, 384]
        ysl = slice(128 * c, 128 * c + 128)
        WpT = np.ascontiguousarray(W_proj[:, ysl].T)         # [128, C]
        ApT = np.ascontiguousarray(ApT_full[:, ysl].T)       # [128, RANK]
        h = np.float16
        in_maps.append({
            "xT": xT.astype(h), "Wq": WqT.astype(h), "bq": b_sl,
            "Aq": AqT.astype(h), "Bq": Bq_s.astype(h), "Wp": WpT.astype(h),
            "Ap": ApT.astype(h), "Bp": Bp_s.astype(h), "tri": tri.astype(h),
            "ones64": ones64.astype(h), "eye": eye.astype(h),
            "onesb": np.ones((128, 16), h),
        })
    return in_maps


def _install_ntff_shim():
    """Provide antenv.axon_hooks (missing on this image) via ctypes against
    the axon .so, mirroring trn_agent_boot.trn_boot._ntff_profile_via_ctypes."""
    import types
    import ctypes
    import contextlib
    try:
        from antenv.axon_hooks import get_axon_ntff_profile_hook  # noqa: F401
        return
    except ImportError:
        pass
    so_path = "/opt/axon/libaxon_pjrt.so"
    try:
        lib = ctypes.CDLL(so_path)
    except OSError:
        return
    if not hasattr(lib, "axon_start_nrt_profile"):
        return
    lib.axon_start_nrt_profile.argtypes = [ctypes.POINTER(ctypes.c_int64),
                                           ctypes.c_size_t]
    lib.axon_start_nrt_profile.restype = ctypes.c_int64
    lib.axon_stop_nrt_profile.argtypes = [ctypes.c_char_p]
    lib.axon_stop_nrt_profile.restype = ctypes.c_int64

    @contextlib.contextmanager
    def _hook(output_dir, device_ids):
        import jax
        jax.devices()
        if device_ids:
            ids = (ctypes.c_int64 * len(device_ids))(*device_ids)
            rc = lib.axon_start_nrt_profile(ids, len(device_ids))
        else:
            rc = lib.axon_start_nrt_profile(None, 0)
        if rc != 0:
            raise RuntimeError(f"axon_start_nrt_profile rc={rc}")
        try:
            yield
        finally:
            n = lib.axon_stop_nrt_profile(str(output_dir).encode())
            if n < 0:
                raise RuntimeError(f"axon_stop_nrt_profile rc={n}")

    import antenv
    mod = types.ModuleType("antenv.axon_hooks")
    mod.get_axon_ntff_profile_hook = lambda: _hook
    mod.set_axon_ntff_profile_hook = lambda h: None
    sys.modules["antenv.axon_hooks"] = mod
    antenv.axon_hooks = mod


def run(inputs, trace=False, trace_cores=None):
    """Run the kernel. Returns (output, BassKernelResults)."""
    if "nc" not in _cache:
        _cache["nc"] = _build()
    nc = _cache["nc"]
    inputs = {k: np.asarray(v, dtype=np.float32) for k, v in inputs.items()}
    in_maps = _prep_inputs(**inputs)
    if trace:
        _install_ntff_shim()
    res = run_bass_kernel_spmd(nc, in_maps, core_ids=list(range(NCORES)),
                               trace=trace, trace_cores=trace_cores)
    outT = np.zeros((C, BT), np.float64)
    for r in res.results:
        outT += r["outT"].astype(np.float64)
    out = outT.T + inputs["b_proj"][None, :]
    return out.astype(np.float32).reshape(B, T, C), res


def kernel(**inputs):
    out, _ = run(inputs, trace=False)
    return out


# revision 11
# speedup vs baseline: 1.2601x; 1.0921x over previous
"""Trainium2 Bass kernel for causal multi-head attention with LoRA (QKV + proj).

Problem (hardcoded): B=4, T=2048, C=1024, NH=16, HD=64, RANK=56, alpha=8.

Sharding: tensor-parallel across heads — each of the 8 cores owns 2 heads
(128 qkv dims per projection) and processes all 4 batches. The output
projection is row-parallel (each core contracts over its own 128 y dims);
partial outputs are summed on the host.

All matmuls run in float32r (TF32-like, ~1e-4 relative rounding, full PE
rate for moving dims >= 256). Layout is transposed throughout: activations
live as [feature(partition), token(free)], which makes the QKV projection,
QK^T, PV and output projection all natural matmuls. The only on-chip
transposes are V (needed token-major for PV): 16 PE transposes per batch.

Softmax: scoresT [tk, tq] -> exp on ACT (scale=1/8 folded in); causal
masking via block-sliced matmul ranges + one [128,128] triangular mask
multiply per diagonal block; row sums via an appended ones column in the
PV stationary operand; normalization via reciprocal + K=1 broadcast matmul.
"""
import sys
import numpy as np

if "/opt/trn_rl_repo" not in sys.path:
    sys.path.insert(0, "/opt/trn_rl_repo")

import concourse.bass as bass  # noqa: E402
from concourse import bacc  # noqa: E402
import concourse.mybir as mybir  # noqa: E402
import concourse.tile as tile  # noqa: E402
from concourse.bass_utils import run_bass_kernel_spmd  # noqa: E402

B, T, C = 4, 2048, 1024
NH, HD, RANK = 16, 64, 56
SCALING = 8.0 / 56.0
NCORES = 8
BT = B * T            # 8192
TOK = 512             # token chunk (matmul moving dim)
NT4 = T // TOK        # 4 token chunks per batch
NCIN = C // 128       # 8 input-feature chunks
NCO = C // 128        # 8 output-feature chunks (proj)
F32 = mybir.dt.float32
F32R = mybir.dt.float32r
F16 = mybir.dt.float16
EXPF = mybir.ActivationFunctionType.Exp
COPYF = mybir.ActivationFunctionType.Identity  # Copy rejects AP bias

_cache = {}


def _build():
    nc = bacc.Bacc("TRN2", target_bir_lowering=False, debug=False,
                   num_devices=NCORES)
    xT = nc.dram_tensor("xT", [C, BT], F16, kind="ExternalInput")
    Wq = nc.dram_tensor("Wq", [NCIN, 128, 384], F16, kind="ExternalInput")
    bq = nc.dram_tensor("bq", [128, 3], F32, kind="ExternalInput")
    Aq = nc.dram_tensor("Aq", [NCIN, 128, RANK], F16, kind="ExternalInput")
    Bq = nc.dram_tensor("Bq", [RANK, 384], F16, kind="ExternalInput")
    Wp = nc.dram_tensor("Wp", [128, C], F16, kind="ExternalInput")
    Ap = nc.dram_tensor("Ap", [128, RANK], F16, kind="ExternalInput")
    Bp = nc.dram_tensor("Bp", [RANK, C], F16, kind="ExternalInput")
    tri = nc.dram_tensor("tri", [128, 128], F16, kind="ExternalInput")
    ones64 = nc.dram_tensor("ones64", [1, 64], F16, kind="ExternalInput")
    eye = nc.dram_tensor("eye", [128, 128], F16, kind="ExternalInput")
    onesb = nc.dram_tensor("onesb", [128, 16], F16, kind="ExternalInput")
    outT = nc.dram_tensor("outT", [C, BT], F16, kind="ExternalOutput")

    with tile.TileContext(nc) as tc:
        with (
            tc.tile_pool(name="consts", bufs=1) as consts,
            tc.tile_pool(name="qkv", bufs=2) as qkvp,
            tc.tile_pool(name="vaugp", bufs=2) as vaugp,
            tc.tile_pool(name="ytp", bufs=2) as ytp,
            tc.tile_pool(name="xtp", bufs=16) as xtp,
            tc.tile_pool(name="expp", bufs=8) as expp,
            tc.tile_pool(name="small", bufs=2) as small,
            tc.tile_pool(name="ps", bufs=1, space="PSUM") as ps,
        ):
            wq_sb = consts.tile([128, NCIN, 384], F16)
            nc.sync.dma_start(wq_sb[:], Wq[:].rearrange("c p f -> p c f"))
            aq_sb = consts.tile([128, NCIN, RANK], F16)
            nc.sync.dma_start(aq_sb[:], Aq[:].rearrange("c p f -> p c f"))
            lb_sb = consts.tile([RANK, 384], F16)
            nc.sync.dma_start(lb_sb[:], Bq[:])
            bias_sb = consts.tile([128, 3], F32)
            nc.sync.dma_start(bias_sb[:], bq[:])
            wp_sb = consts.tile([128, C], F16)
            nc.sync.dma_start(wp_sb[:], Wp[:])
            ap_sb = consts.tile([128, RANK], F16)
            nc.sync.dma_start(ap_sb[:], Ap[:])
            pb_sb = consts.tile([RANK, C], F16)
            nc.sync.dma_start(pb_sb[:], Bp[:])
            tri_sb = consts.tile([128, 128], F16)
            nc.sync.dma_start(tri_sb[:], tri[:])
            ones_sb = consts.tile([1, 64], F16)
            nc.sync.dma_start(ones_sb[:], ones64[:])
            eye_sb = consts.tile([128, 128], F16)
            nc.sync.dma_start(eye_sb[:], eye[:])

            for b in range(B):
                # ---- QKV projection (+ LoRA) for batch b ----
                qT = qkvp.tile([128, T], F16, tag="qT")
                kT = qkvp.tile([128, T], F16, tag="kT")
                vT = qkvp.tile([128, T], F16, tag="vT")
                with nc.named_scope(f"qkv{b}"):
                    for t4 in range(NT4):
                        gcol = b * T + t4 * TOK
                        xts = []
                        for cc in range(NCIN):
                            xt = xtp.tile([128, TOK], F16, tag="xt")
                            nc.sync.dma_start(
                                xt[:],
                                xT[cc * 128:(cc + 1) * 128, gcol:gcol + TOK])
                            xts.append(xt)
                        ps_r = ps.tile([RANK, TOK], F32, tag="accr", bufs=1)
                        for cc in range(NCIN):
                            nc.tensor.matmul(ps_r[:], aq_sb[:, cc, :], xts[cc][:],
                                             start=(cc == 0), stop=(cc == NCIN - 1))
                        r_sb = small.tile([RANK, TOK], F16, tag="r")
                        nc.vector.tensor_copy(r_sb[:], ps_r[:])
                        for ch, dest in enumerate((qT, kT, vT)):
                            ps_q = ps.tile([128, TOK], F32, tag="acc", bufs=2)
                            for cc in range(NCIN):
                                nc.tensor.matmul(
                                    ps_q[:],
                                    wq_sb[:, cc, ch * 128:(ch + 1) * 128],
                                    xts[cc][:], start=(cc == 0), stop=False)
                            nc.tensor.matmul(
                                ps_q[:], lb_sb[:, ch * 128:(ch + 1) * 128],
                                r_sb[:], start=False, stop=True)
                            nc.scalar.activation(
                                out=dest[:, t4 * TOK:(t4 + 1) * TOK],
                                in_=ps_q[:], func=COPYF,
                                bias=bias_sb[:, ch:ch + 1], scale=1.0)

                # ---- V -> token-major (+ ones cols) ----
                vaug = vaugp.tile([128, 16, 130], F16, tag="vaug")
                with nc.named_scope(f"vtr{b}"):
                    nc.sync.dma_start(vaug[:, :, 64:65],
                                      onesb[:].unsqueeze(-1))
                    nc.sync.dma_start(vaug[:, :, 129:130],
                                      onesb[:].unsqueeze(-1))
                    for tb in range(16):
                        ps_t = ps.tile([128, 128], F16, tag="mm", bufs=3)
                        nc.tensor.transpose(ps_t[:], vT[:, tb * 128:(tb + 1) * 128],
                                            eye_sb[:])
                        nc.vector.tensor_copy(vaug[:, tb, 0:64], ps_t[:, 0:64])
                        nc.vector.tensor_copy(vaug[:, tb, 65:129], ps_t[:, 64:128])

                # ---- attention (both heads interleaved for PE packing) ----
                yt = ytp.tile([128, T], F16, tag="yt")
                with nc.named_scope(f"attn{b}"):
                    for t4 in range(NT4):
                        nblk = 4 * (t4 + 1)
                        q0s, exps = {}, {}
                        psy0 = ps.tile([65, TOK], F32, tag="accy", bufs=2)
                        psy1 = ps.tile([65, TOK], F32, tag="accy", bufs=2)
                        psy = {0: psy0, 1: psy1}

                        def emit_qk(j, h, t4=t4, q0s=q0s, exps=exps):
                            hp = h * 64
                            r = j - 4 * t4
                            q0 = 128 * r if r > 0 else 0
                            q0s[j] = q0
                            ps_s = ps.tile([128, TOK], F32, tag="mm", bufs=3)
                            nc.tensor.matmul(
                                ps_s[:, q0:TOK],
                                kT[hp:hp + 64, j * 128:(j + 1) * 128],
                                qT[hp:hp + 64, t4 * TOK + q0:(t4 + 1) * TOK],
                                start=True, stop=True)
                            e = expp.tile([128, TOK], F16, tag="expS")
                            nc.scalar.activation(
                                out=e[:, q0:TOK], in_=ps_s[:, q0:TOK],
                                func=EXPF, scale=0.125)
                            if r >= 0:
                                nc.vector.tensor_mul(
                                    e[:, q0:q0 + 128], e[:, q0:q0 + 128],
                                    tri_sb[:])
                            exps[(j, h)] = e

                        def emit_pv(j, h, nblk=nblk, q0s=q0s, exps=exps,
                                    psy=psy, vaug=vaug):
                            q0 = q0s[j]
                            nc.tensor.matmul(
                                psy[h][:, q0:TOK],
                                vaug[:, j, 65 * h:65 * h + 65],
                                exps.pop((j, h))[:, q0:TOK],
                                start=(j == 0), stop=(j == nblk - 1))

                        for h in (0, 1):
                            emit_qk(0, h)
                        if nblk > 1:
                            for h in (0, 1):
                                emit_qk(1, h)
                        for j in range(nblk):
                            if j + 2 < nblk:
                                for h in (0, 1):
                                    emit_qk(j + 2, h)
                            for h in (0, 1):
                                emit_pv(j, h)

                        tsl = slice(t4 * TOK, (t4 + 1) * TOK)
                        for h in (0, 1):
                            zrow = small.tile([65, TOK], F32, tag="zrow")
                            nc.vector.tensor_copy(zrow[64:65, :],
                                                  psy[h][64:65, :])
                            z0 = small.tile([1, TOK], F32, tag="z0")
                            nc.sync.dma_start(z0[:], zrow[64:65, :])
                            recipf = small.tile([1, TOK], F32, tag="recipf")
                            nc.vector.reciprocal_approx_fast(
                                out=recipf[:], in_=z0[:])
                            recip0 = small.tile([1, TOK], F16, tag="recip0")
                            # gpsimd DMA casts f32->f16
                            nc.gpsimd.dma_start(recip0[:], recipf[:])
                            ps_b = ps.tile([64, TOK], F32, tag="mm", bufs=3)
                            nc.tensor.matmul(ps_b[:], ones_sb[:], recip0[:],
                                             start=True, stop=True)
                            sb_b = small.tile([64, TOK], F16, tag="sbb")
                            nc.vector.tensor_copy(sb_b[:], ps_b[:])
                            if h == 0:
                                nc.vector.tensor_mul(yt[0:64, tsl],
                                                     psy[0][0:64, :], sb_b[:])
                            else:
                                stage = small.tile([64, TOK], F16, tag="stage")
                                nc.vector.tensor_mul(stage[:], psy[1][0:64, :],
                                                     sb_b[:])
                                nc.sync.dma_start(yt[64:128, tsl], stage[:])

                # ---- output projection (+ LoRA), row-parallel partial ----
                with nc.named_scope(f"proj{b}"):
                    for t4 in range(NT4):
                        gcol = b * T + t4 * TOK
                        tsl = slice(t4 * TOK, (t4 + 1) * TOK)
                        ps_rp = ps.tile([RANK, TOK], F32, tag="accr", bufs=1)
                        nc.tensor.matmul(ps_rp[:], ap_sb[:], yt[:, tsl],
                                         start=True, stop=True)
                        rp_sb = small.tile([RANK, TOK], F16, tag="rp")
                        nc.vector.tensor_copy(rp_sb[:], ps_rp[:])
                        for co in range(NCO):
                            ps_o = ps.tile([128, TOK], F32, tag="acc", bufs=2)
                            nc.tensor.matmul(
                                ps_o[:], wp_sb[:, co * 128:(co + 1) * 128],
                                yt[:, tsl], start=True, stop=False)
                            nc.tensor.matmul(
                                ps_o[:], pb_sb[:, co * 128:(co + 1) * 128],
                                rp_sb[:], start=False, stop=True)
                            po = small.tile([128, TOK], F16, tag="po", bufs=3)
                            nc.any.tensor_copy(po[:], ps_o[:])
                            nc.sync.dma_start(
                                outT[co * 128:(co + 1) * 128, gcol:gcol + TOK],
                                po[:])
    nc.compile()
    return nc


def _prep_inputs(x, W_attn, b_attn, A_attn, B_attn, W_proj, b_proj, A_proj,
                 B_proj):
    xT = np.ascontiguousarray(x.reshape(BT, C).T)
    AqT = np.ascontiguousarray(A_attn.T).reshape(NCIN, 128, RANK)
    ApT_full = A_proj  # [RANK, C]
    tri = np.triu(np.ones((128, 128), np.float32))
    ones64 = np.ones((1, 64), np.float32)
    eye = np.eye(128, dtype=np.float32)
    Bp_s = np.ascontiguousarray((B_proj * SCALING).T)  # [RANK, C]
    in_maps = []
    for c in range(NCORES):
        rows = np.r_[128 * c:128 * c + 128,
                     C + 128 * c:C + 128 * c + 128,
                     2 * C + 128 * c:2 * C + 128 * c + 128]
        W_sl = W_attn[rows]                                  # [384, C]
        WqT = np.ascontiguousarray(W_sl.T).reshape(NCIN, 128, 384)
        b_sl = np.ascontiguousarray(b_attn[rows].reshape(3, 128).T)
        Bq_s = np.ascontiguousarray((B_attn[rows] * SCALING).T)  # [RANK, 384]
        ysl = slice(128 * c, 128 * c + 128)
        WpT = np.ascontiguousarray(W_proj[:, ysl].T)         # [128, C]
        ApT = np.ascontiguousarray(ApT_full[:, ysl].T)       # [128, RANK]
        h = np.float16
        in_maps.append({
            "xT": xT.astype(h), "Wq": WqT.astype(h), "bq": b_sl,
            "Aq": AqT.astype(h), "Bq": Bq_s.astype(h), "Wp": WpT.astype(h),
            "Ap": ApT.astype(h), "Bp": Bp_s.astype(h), "tri": tri.astype(h),
            "ones64": ones64.astype(h), "eye": eye.astype(h),
            "onesb": np.ones((128, 16), h),
        })
    return in_maps


def _install_ntff_shim():
    """Provide antenv.axon_hooks (missing on this image) via ctypes against
    the axon .so, mirroring trn_agent_boot.trn_boot._ntff_profile_via_ctypes."""
    import types
    import ctypes
    import contextlib
    try:
        from antenv.axon_hooks import get_axon_ntff_profile_hook  # noqa: F401
        return
    except ImportError:
        pass
    so_path = "/opt/axon/libaxon_pjrt.so"
    try:
        lib = ctypes.CDLL(so_path)
    except OSError:
        return
    if not hasattr(lib, "axon_start_nrt_profile"):
        return
    lib.axon_start_nrt_profile.argtypes = [ctypes.POINTER(ctypes.c_int64),
                                           ctypes.c_size_t]
    lib.axon_start_nrt_profile.restype = ctypes.c_int64
    lib.axon_stop_nrt_profile.argtypes = [ctypes.c_char_p]
    lib.axon_stop_nrt_profile.restype = ctypes.c_int64

    @contextlib.contextmanager
    def _hook(output_dir, device_ids):
        import jax
        jax.devices()
        if device_ids:
            ids = (ctypes.c_int64 * len(device_ids))(*device_ids)
            rc = lib.axon_start_nrt_profile(ids, len(device_ids))
        else:
            rc = lib.axon_start_nrt_profile(None, 0)
        if rc != 0:
            raise RuntimeError(f"axon_start_nrt_profile rc={rc}")
        try:
            yield
        finally:
            n = lib.axon_stop_nrt_profile(str(output_dir).encode())
            if n < 0:
                raise RuntimeError(f"axon_stop_nrt_profile rc={n}")

    import antenv
    mod = types.ModuleType("antenv.axon_hooks")
    mod.get_axon_ntff_profile_hook = lambda: _hook
    mod.set_axon_ntff_profile_hook = lambda h: None
    sys.modules["antenv.axon_hooks"] = mod
    antenv.axon_hooks = mod


def run(inputs, trace=False, trace_cores=None):
    """Run the kernel. Returns (output, BassKernelResults)."""
    if "nc" not in _cache:
        _cache["nc"] = _build()
    nc = _cache["nc"]
    inputs = {k: np.asarray(v, dtype=np.float32) for k, v in inputs.items()}
    in_maps = _prep_inputs(**inputs)
    if trace:
        _install_ntff_shim()
    res = run_bass_kernel_spmd(nc, in_maps, core_ids=list(range(NCORES)),
                               trace=trace, trace_cores=trace_cores)
    outT = np.zeros((C, BT), np.float32)
    for r in res.results:
        outT += r["outT"].astype(np.float32)
    out = outT.T + inputs["b_proj"][None, :]
    return out.astype(np.float32).reshape(B, T, C), res


def kernel(**inputs):
    out, _ = run(inputs, trace=False)
    return out


# revision 12
# speedup vs baseline: 1.2619x; 1.0014x over previous
"""Trainium2 Bass kernel for causal multi-head attention with LoRA (QKV + proj).

Problem (hardcoded): B=4, T=2048, C=1024, NH=16, HD=64, RANK=56, alpha=8.

Sharding: tensor-parallel across heads — each of the 8 cores owns 2 heads
(128 qkv dims per projection) and processes all 4 batches. The output
projection is row-parallel (each core contracts over its own 128 y dims);
partial outputs are summed on the host.

All matmuls run in float32r (TF32-like, ~1e-4 relative rounding, full PE
rate for moving dims >= 256). Layout is transposed throughout: activations
live as [feature(partition), token(free)], which makes the QKV projection,
QK^T, PV and output projection all natural matmuls. The only on-chip
transposes are V (needed token-major for PV): 16 PE transposes per batch.

Softmax: scoresT [tk, tq] -> exp on ACT (scale=1/8 folded in); causal
masking via block-sliced matmul ranges + one [128,128] triangular mask
multiply per diagonal block; row sums via an appended ones column in the
PV stationary operand; normalization via reciprocal + K=1 broadcast matmul.
"""
import sys
import numpy as np

if "/opt/trn_rl_repo" not in sys.path:
    sys.path.insert(0, "/opt/trn_rl_repo")

import concourse.bass as bass  # noqa: E402
from concourse import bacc  # noqa: E402
import concourse.mybir as mybir  # noqa: E402
import concourse.tile as tile  # noqa: E402
from concourse.bass_utils import run_bass_kernel_spmd  # noqa: E402

B, T, C = 4, 2048, 1024
NH, HD, RANK = 16, 64, 56
SCALING = 8.0 / 56.0
NCORES = 8
BT = B * T            # 8192
TOK = 512             # token chunk (matmul moving dim)
NT4 = T // TOK        # 4 token chunks per batch
NCIN = C // 128       # 8 input-feature chunks
NCO = C // 128        # 8 output-feature chunks (proj)
F32 = mybir.dt.float32
F32R = mybir.dt.float32r
F16 = mybir.dt.float16
EXPF = mybir.ActivationFunctionType.Exp
COPYF = mybir.ActivationFunctionType.Identity  # Copy rejects AP bias

_cache = {}


def _build():
    nc = bacc.Bacc("TRN2", target_bir_lowering=False, debug=False,
                   num_devices=NCORES)
    xT = nc.dram_tensor("xT", [C, BT], F16, kind="ExternalInput")
    Wq = nc.dram_tensor("Wq", [NCIN, 128, 384], F16, kind="ExternalInput")
    bq = nc.dram_tensor("bq", [128, 3], F32, kind="ExternalInput")
    Aq = nc.dram_tensor("Aq", [NCIN, 128, RANK], F16, kind="ExternalInput")
    Bq = nc.dram_tensor("Bq", [RANK, 384], F16, kind="ExternalInput")
    Wp = nc.dram_tensor("Wp", [128, C], F16, kind="ExternalInput")
    Ap = nc.dram_tensor("Ap", [128, RANK], F16, kind="ExternalInput")
    Bp = nc.dram_tensor("Bp", [RANK, C], F16, kind="ExternalInput")
    tri = nc.dram_tensor("tri", [128, 128], F16, kind="ExternalInput")
    ones64 = nc.dram_tensor("ones64", [1, 64], F16, kind="ExternalInput")
    eye = nc.dram_tensor("eye", [128, 128], F16, kind="ExternalInput")
    onesb = nc.dram_tensor("onesb", [128, 16], F16, kind="ExternalInput")
    outT = nc.dram_tensor("outT", [C, BT], F16, kind="ExternalOutput")

    with tile.TileContext(nc) as tc:
        with (
            tc.tile_pool(name="consts", bufs=1) as consts,
            tc.tile_pool(name="qkv", bufs=2) as qkvp,
            tc.tile_pool(name="vaugp", bufs=2) as vaugp,
            tc.tile_pool(name="ytp", bufs=2) as ytp,
            tc.tile_pool(name="xtp", bufs=16) as xtp,
            tc.tile_pool(name="expp", bufs=8) as expp,
            tc.tile_pool(name="small", bufs=2) as small,
            tc.tile_pool(name="ps", bufs=1, space="PSUM") as ps,
        ):
            wq_sb = consts.tile([128, NCIN, 384], F16)
            nc.sync.dma_start(wq_sb[:], Wq[:].rearrange("c p f -> p c f"))
            aq_sb = consts.tile([128, NCIN, RANK], F16)
            nc.sync.dma_start(aq_sb[:], Aq[:].rearrange("c p f -> p c f"))
            lb_sb = consts.tile([RANK, 384], F16)
            nc.sync.dma_start(lb_sb[:], Bq[:])
            bias_sb = consts.tile([128, 3], F32)
            nc.sync.dma_start(bias_sb[:], bq[:])
            wp_sb = consts.tile([128, C], F16)
            nc.sync.dma_start(wp_sb[:], Wp[:])
            ap_sb = consts.tile([128, RANK], F16)
            nc.sync.dma_start(ap_sb[:], Ap[:])
            pb_sb = consts.tile([RANK, C], F16)
            nc.sync.dma_start(pb_sb[:], Bp[:])
            tri_sb = consts.tile([128, 128], F16)
            nc.sync.dma_start(tri_sb[:], tri[:])
            ones_sb = consts.tile([1, 64], F16)
            nc.sync.dma_start(ones_sb[:], ones64[:])
            eye_sb = consts.tile([128, 128], F16)
            nc.sync.dma_start(eye_sb[:], eye[:])

            for b in range(B):
                # ---- QKV projection (+ LoRA) for batch b ----
                qT = qkvp.tile([128, T], F16, tag="qT")
                kT = qkvp.tile([128, T], F16, tag="kT")
                vT = qkvp.tile([128, T], F16, tag="vT")
                with nc.named_scope(f"qkv{b}"):
                    for t4 in range(NT4):
                        gcol = b * T + t4 * TOK
                        xts = []
                        for cc in range(NCIN):
                            xt = xtp.tile([128, TOK], F16, tag="xt")
                            nc.sync.dma_start(
                                xt[:],
                                xT[cc * 128:(cc + 1) * 128, gcol:gcol + TOK])
                            xts.append(xt)
                        ps_r = ps.tile([RANK, TOK], F32, tag="accr", bufs=1)
                        for cc in range(NCIN):
                            nc.tensor.matmul(ps_r[:], aq_sb[:, cc, :], xts[cc][:],
                                             start=(cc == 0), stop=(cc == NCIN - 1))
                        r_sb = small.tile([RANK, TOK], F16, tag="r")
                        nc.vector.tensor_copy(r_sb[:], ps_r[:])
                        for ch, dest in enumerate((qT, kT, vT)):
                            ps_q = ps.tile([128, TOK], F32, tag="acc", bufs=2)
                            for cc in range(NCIN):
                                nc.tensor.matmul(
                                    ps_q[:],
                                    wq_sb[:, cc, ch * 128:(ch + 1) * 128],
                                    xts[cc][:], start=(cc == 0), stop=False)
                            nc.tensor.matmul(
                                ps_q[:], lb_sb[:, ch * 128:(ch + 1) * 128],
                                r_sb[:], start=False, stop=True)
                            nc.scalar.activation(
                                out=dest[:, t4 * TOK:(t4 + 1) * TOK],
                                in_=ps_q[:], func=COPYF,
                                bias=bias_sb[:, ch:ch + 1], scale=1.0)

                # ---- V -> token-major (+ ones cols) ----
                vaug = vaugp.tile([128, 16, 132], F16, tag="vaug")
                with nc.named_scope(f"vtr{b}"):
                    nc.sync.dma_start(vaug[:, :, 64:65],
                                      onesb[:].unsqueeze(-1))
                    nc.sync.dma_start(vaug[:, :, 130:131],
                                      onesb[:].unsqueeze(-1))
                    for tb in range(16):
                        ps_t = ps.tile([128, 128], F16, tag="mm", bufs=3)
                        nc.tensor.transpose(ps_t[:], vT[:, tb * 128:(tb + 1) * 128],
                                            eye_sb[:])
                        nc.vector.tensor_copy(vaug[:, tb, 0:64], ps_t[:, 0:64])
                        nc.vector.tensor_copy(vaug[:, tb, 66:130], ps_t[:, 64:128])

                # ---- attention (both heads interleaved for PE packing) ----
                yt = ytp.tile([128, T], F16, tag="yt")
                with nc.named_scope(f"attn{b}"):
                    for t4 in range(NT4):
                        nblk = 4 * (t4 + 1)
                        q0s, exps = {}, {}
                        psy0 = ps.tile([65, TOK], F32, tag="accy", bufs=2)
                        psy1 = ps.tile([65, TOK], F32, tag="accy", bufs=2)
                        psy = {0: psy0, 1: psy1}

                        def emit_qk(j, h, t4=t4, q0s=q0s, exps=exps):
                            hp = h * 64
                            r = j - 4 * t4
                            q0 = 128 * r if r > 0 else 0
                            q0s[j] = q0
                            ps_s = ps.tile([128, TOK], F32, tag="mm", bufs=3)
                            nc.tensor.matmul(
                                ps_s[:, q0:TOK],
                                kT[hp:hp + 64, j * 128:(j + 1) * 128],
                                qT[hp:hp + 64, t4 * TOK + q0:(t4 + 1) * TOK],
                                start=True, stop=True,
                                tile_position=(hp, 0))
                            e = expp.tile([128, TOK], F16, tag="expS")
                            nc.scalar.activation(
                                out=e[:, q0:TOK], in_=ps_s[:, q0:TOK],
                                func=EXPF, scale=0.125)
                            if r >= 0:
                                nc.vector.tensor_mul(
                                    e[:, q0:q0 + 128], e[:, q0:q0 + 128],
                                    tri_sb[:])
                            exps[(j, h)] = e

                        def emit_pv(j, h, nblk=nblk, q0s=q0s, exps=exps,
                                    psy=psy, vaug=vaug):
                            q0 = q0s[j]
                            nc.tensor.matmul(
                                psy[h][:, q0:TOK],
                                vaug[:, j, 66 * h:66 * h + 65],
                                exps.pop((j, h))[:, q0:TOK],
                                start=(j == 0), stop=(j == nblk - 1))

                        for h in (0, 1):
                            emit_qk(0, h)
                        if nblk > 1:
                            for h in (0, 1):
                                emit_qk(1, h)
                        for j in range(nblk):
                            if j + 2 < nblk:
                                for h in (0, 1):
                                    emit_qk(j + 2, h)
                            for h in (0, 1):
                                emit_pv(j, h)

                        tsl = slice(t4 * TOK, (t4 + 1) * TOK)
                        for h in (0, 1):
                            zrow = small.tile([65, TOK], F32, tag="zrow")
                            nc.vector.tensor_copy(zrow[64:65, :],
                                                  psy[h][64:65, :])
                            z0 = small.tile([1, TOK], F32, tag="z0")
                            nc.sync.dma_start(z0[:], zrow[64:65, :])
                            recipf = small.tile([1, TOK], F32, tag="recipf")
                            nc.vector.reciprocal_approx_fast(
                                out=recipf[:], in_=z0[:])
                            recip0 = small.tile([1, TOK], F16, tag="recip0")
                            # gpsimd DMA casts f32->f16
                            nc.gpsimd.dma_start(recip0[:], recipf[:])
                            ps_b = ps.tile([64, TOK], F32, tag="mm", bufs=3)
                            nc.tensor.matmul(ps_b[:], ones_sb[:], recip0[:],
                                             start=True, stop=True)
                            sb_b = small.tile([64, TOK], F16, tag="sbb")
                            nc.vector.tensor_copy(sb_b[:], ps_b[:])
                            if h == 0:
                                nc.vector.tensor_mul(yt[0:64, tsl],
                                                     psy[0][0:64, :], sb_b[:])
                            else:
                                stage = small.tile([64, TOK], F16, tag="stage")
                                nc.vector.tensor_mul(stage[:], psy[1][0:64, :],
                                                     sb_b[:])
                                nc.sync.dma_start(yt[64:128, tsl], stage[:])

                # ---- output projection (+ LoRA), row-parallel partial ----
                with nc.named_scope(f"proj{b}"):
                    for t4 in range(NT4):
                        gcol = b * T + t4 * TOK
                        tsl = slice(t4 * TOK, (t4 + 1) * TOK)
                        ps_rp = ps.tile([RANK, TOK], F32, tag="accr", bufs=1)
                        nc.tensor.matmul(ps_rp[:], ap_sb[:], yt[:, tsl],
                                         start=True, stop=True)
                        rp_sb = small.tile([RANK, TOK], F16, tag="rp")
                        nc.vector.tensor_copy(rp_sb[:], ps_rp[:])
                        for co in range(NCO):
                            ps_o = ps.tile([128, TOK], F32, tag="acc", bufs=2)
                            nc.tensor.matmul(
                                ps_o[:], wp_sb[:, co * 128:(co + 1) * 128],
                                yt[:, tsl], start=True, stop=False)
                            nc.tensor.matmul(
                                ps_o[:], pb_sb[:, co * 128:(co + 1) * 128],
                                rp_sb[:], start=False, stop=True)
                            po = small.tile([128, TOK], F16, tag="po", bufs=3)
                            nc.any.tensor_copy(po[:], ps_o[:])
                            nc.sync.dma_start(
                                outT[co * 128:(co + 1) * 128, gcol:gcol + TOK],
                                po[:])
    nc.compile()
    return nc


def _prep_inputs(x, W_attn, b_attn, A_attn, B_attn, W_proj, b_proj, A_proj,
                 B_proj):
    xT = np.ascontiguousarray(x.reshape(BT, C).T)
    AqT = np.ascontiguousarray(A_attn.T).reshape(NCIN, 128, RANK)
    ApT_full = A_proj  # [RANK, C]
    tri = np.triu(np.ones((128, 128), np.float32))
    ones64 = np.ones((1, 64), np.float32)
    eye = np.eye(128, dtype=np.float32)
    Bp_s = np.ascontiguousarray((B_proj * SCALING).T)  # [RANK, C]
    in_maps = []
    for c in range(NCORES):
        rows = np.r_[128 * c:128 * c + 128,
                     C + 128 * c:C + 128 * c + 128,
                     2 * C + 128 * c:2 * C + 128 * c + 128]
        W_sl = W_attn[rows]                                  # [384, C]
        WqT = np.ascontiguousarray(W_sl.T).reshape(NCIN, 128, 384)
        b_sl = np.ascontiguousarray(b_attn[rows].reshape(3, 128).T)
        Bq_s = np.ascontiguousarray((B_attn[rows] * SCALING).T)  # [RANK, 384]
        ysl = slice(128 * c, 128 * c + 128)
        WpT = np.ascontiguousarray(W_proj[:, ysl].T)         # [128, C]
        ApT = np.ascontiguousarray(ApT_full[:, ysl].T)       # [128, RANK]
        h = np.float16
        in_maps.append({
            "xT": xT.astype(h), "Wq": WqT.astype(h), "bq": b_sl,
            "Aq": AqT.astype(h), "Bq": Bq_s.astype(h), "Wp": WpT.astype(h),
            "Ap": ApT.astype(h), "Bp": Bp_s.astype(h), "tri": tri.astype(h),
            "ones64": ones64.astype(h), "eye": eye.astype(h),
            "onesb": np.ones((128, 16), h),
        })
    return in_maps


def _install_ntff_shim():
    """Provide antenv.axon_hooks (missing on this image) via ctypes against
    the axon .so, mirroring trn_agent_boot.trn_boot._ntff_profile_via_ctypes."""
    import types
    import ctypes
    import contextlib
    try:
        from antenv.axon_hooks import get_axon_ntff_profile_hook  # noqa: F401
        return
    except ImportError:
        pass
    so_path = "/opt/axon/libaxon_pjrt.so"
    try:
        lib = ctypes.CDLL(so_path)
    except OSError:
        return
    if not hasattr(lib, "axon_start_nrt_profile"):
        return
    lib.axon_start_nrt_profile.argtypes = [ctypes.POINTER(ctypes.c_int64),
                                           ctypes.c_size_t]
    lib.axon_start_nrt_profile.restype = ctypes.c_int64
    lib.axon_stop_nrt_profile.argtypes = [ctypes.c_char_p]
    lib.axon_stop_nrt_profile.restype = ctypes.c_int64

    @contextlib.contextmanager
    def _hook(output_dir, device_ids):
        import jax
        jax.devices()
        if device_ids:
            ids = (ctypes.c_int64 * len(device_ids))(*device_ids)
            rc = lib.axon_start_nrt_profile(ids, len(device_ids))
        else:
            rc = lib.axon_start_nrt_profile(None, 0)
        if rc != 0:
            raise RuntimeError(f"axon_start_nrt_profile rc={rc}")
        try:
            yield
        finally:
            n = lib.axon_stop_nrt_profile(str(output_dir).encode())
            if n < 0:
                raise RuntimeError(f"axon_stop_nrt_profile rc={n}")

    import antenv
    mod = types.ModuleType("antenv.axon_hooks")
    mod.get_axon_ntff_profile_hook = lambda: _hook
    mod.set_axon_ntff_profile_hook = lambda h: None
    sys.modules["antenv.axon_hooks"] = mod
    antenv.axon_hooks = mod


def run(inputs, trace=False, trace_cores=None):
    """Run the kernel. Returns (output, BassKernelResults)."""
    if "nc" not in _cache:
        _cache["nc"] = _build()
    nc = _cache["nc"]
    inputs = {k: np.asarray(v, dtype=np.float32) for k, v in inputs.items()}
    in_maps = _prep_inputs(**inputs)
    if trace:
        _install_ntff_shim()
    res = run_bass_kernel_spmd(nc, in_maps, core_ids=list(range(NCORES)),
                               trace=trace, trace_cores=trace_cores)
    outT = np.zeros((C, BT), np.float32)
    for r in res.results:
        outT += r["outT"].astype(np.float32)
    out = outT.T + inputs["b_proj"][None, :]
    return out.astype(np.float32).reshape(B, T, C), res


def kernel(**inputs):
    out, _ = run(inputs, trace=False)
    return out


# revision 13
# speedup vs baseline: 1.4938x; 1.1838x over previous
"""Trainium2 Bass kernel for causal multi-head attention with LoRA (QKV + proj).

Problem (hardcoded): B=4, T=2048, C=1024, NH=16, HD=64, RANK=56, alpha=8.

Sharding: tensor-parallel across heads — each of the 8 cores owns 2 heads
(128 qkv dims per projection) and processes all 4 batches. The output
projection is row-parallel (each core contracts over its own 128 y dims);
partial outputs are summed on the host.

All matmuls run in float32r (TF32-like, ~1e-4 relative rounding, full PE
rate for moving dims >= 256). Layout is transposed throughout: activations
live as [feature(partition), token(free)], which makes the QKV projection,
QK^T, PV and output projection all natural matmuls. The only on-chip
transposes are V (needed token-major for PV): 16 PE transposes per batch.

Softmax: scoresT [tk, tq] -> exp on ACT (scale=1/8 folded in); causal
masking via block-sliced matmul ranges + one [128,128] triangular mask
multiply per diagonal block; row sums via an appended ones column in the
PV stationary operand; normalization via reciprocal + K=1 broadcast matmul.
"""
import sys
import numpy as np

if "/opt/trn_rl_repo" not in sys.path:
    sys.path.insert(0, "/opt/trn_rl_repo")

import concourse.bass as bass  # noqa: E402
from concourse import bacc  # noqa: E402
import concourse.mybir as mybir  # noqa: E402
import concourse.tile as tile  # noqa: E402
from concourse.bass_utils import run_bass_kernel_spmd  # noqa: E402

B, T, C = 4, 2048, 1024
NH, HD, RANK = 16, 64, 56
SCALING = 8.0 / 56.0
NCORES = 8
BT = B * T            # 8192
TOK = 512             # token chunk (matmul moving dim)
NT4 = T // TOK        # 4 token chunks per batch
NCIN = C // 128       # 8 input-feature chunks
NCO = C // 128        # 8 output-feature chunks (proj)
F32 = mybir.dt.float32
F32R = mybir.dt.float32r
F16 = mybir.dt.float16
EXPF = mybir.ActivationFunctionType.Exp
COPYF = mybir.ActivationFunctionType.Identity  # Copy rejects AP bias

_cache = {}


def _build():
    nc = bacc.Bacc("TRN2", target_bir_lowering=False, debug=False,
                   num_devices=NCORES)
    xT = nc.dram_tensor("xT", [C, BT], F16, kind="ExternalInput")
    Wq = nc.dram_tensor("Wq", [NCIN, 128, 384], F16, kind="ExternalInput")
    bq = nc.dram_tensor("bq", [128, 3], F32, kind="ExternalInput")
    Aq = nc.dram_tensor("Aq", [NCIN, 128, RANK], F16, kind="ExternalInput")
    Bq = nc.dram_tensor("Bq", [RANK, 384], F16, kind="ExternalInput")
    Wp = nc.dram_tensor("Wp", [128, C], F16, kind="ExternalInput")
    Ap = nc.dram_tensor("Ap", [128, RANK], F16, kind="ExternalInput")
    Bp = nc.dram_tensor("Bp", [RANK, C], F16, kind="ExternalInput")
    tri = nc.dram_tensor("tri", [128, 128], F16, kind="ExternalInput")
    ones64 = nc.dram_tensor("ones64", [1, 64], F16, kind="ExternalInput")
    eye = nc.dram_tensor("eye", [128, 128], F16, kind="ExternalInput")
    onesb = nc.dram_tensor("onesb", [128, 16], F16, kind="ExternalInput")
    outT = nc.dram_tensor("outT", [C, BT], F16, kind="ExternalOutput")

    with tile.TileContext(nc) as tc:
        with (
            tc.tile_pool(name="consts", bufs=1) as consts,
            tc.tile_pool(name="qkv", bufs=2) as qkvp,
            tc.tile_pool(name="vaugp", bufs=2) as vaugp,
            tc.tile_pool(name="ytp", bufs=2) as ytp,
            tc.tile_pool(name="xtp", bufs=16) as xtp,
            tc.tile_pool(name="expp", bufs=8) as expp,
            tc.tile_pool(name="small", bufs=2) as small,
            tc.tile_pool(name="ps", bufs=1, space="PSUM") as ps,
        ):
            wq_sb = consts.tile([128, NCIN, 384], F16)
            nc.sync.dma_start(wq_sb[:], Wq[:].rearrange("c p f -> p c f"))
            aq_sb = consts.tile([128, NCIN, RANK], F16)
            nc.sync.dma_start(aq_sb[:], Aq[:].rearrange("c p f -> p c f"))
            lb_sb = consts.tile([RANK, 384], F16)
            nc.sync.dma_start(lb_sb[:], Bq[:])
            bias_sb = consts.tile([128, 3], F32)
            nc.sync.dma_start(bias_sb[:], bq[:])
            wp_sb = consts.tile([128, C], F16)
            nc.sync.dma_start(wp_sb[:], Wp[:])
            ap_sb = consts.tile([128, RANK], F16)
            nc.sync.dma_start(ap_sb[:], Ap[:])
            pb_sb = consts.tile([RANK, C], F16)
            nc.sync.dma_start(pb_sb[:], Bp[:])
            tri_sb = consts.tile([128, 128], F16)
            nc.sync.dma_start(tri_sb[:], tri[:])
            ones_sb = consts.tile([1, 64], F16)
            nc.sync.dma_start(ones_sb[:], ones64[:])
            eye_sb = consts.tile([128, 128], F16)
            nc.sync.dma_start(eye_sb[:], eye[:])

            for b in range(B):
                # ---- QKV projection (+ LoRA) for batch b ----
                qT = qkvp.tile([128, T], F16, tag="qT")
                kT = qkvp.tile([128, T], F16, tag="kT")
                vT = qkvp.tile([128, T], F16, tag="vT")
                with nc.named_scope(f"qkv{b}"):
                    for t4 in range(NT4):
                        gcol = b * T + t4 * TOK
                        xts = []
                        for cc in range(NCIN):
                            xt = xtp.tile([128, TOK], F16, tag="xt")
                            nc.sync.dma_start(
                                xt[:],
                                xT[cc * 128:(cc + 1) * 128, gcol:gcol + TOK])
                            xts.append(xt)
                        ps_r = ps.tile([RANK, TOK], F32, tag="accr", bufs=1)
                        for cc in range(NCIN):
                            nc.tensor.matmul(ps_r[:], aq_sb[:, cc, :], xts[cc][:],
                                             start=(cc == 0), stop=(cc == NCIN - 1))
                        r_sb = small.tile([RANK, TOK], F16, tag="r")
                        nc.vector.tensor_copy(r_sb[:], ps_r[:])
                        for ch, dest in enumerate((qT, kT, vT)):
                            ps_q = ps.tile([128, TOK], F32, tag="acc", bufs=2)
                            for cc in range(NCIN):
                                nc.tensor.matmul(
                                    ps_q[:],
                                    wq_sb[:, cc, ch * 128:(ch + 1) * 128],
                                    xts[cc][:], start=(cc == 0), stop=False)
                            nc.tensor.matmul(
                                ps_q[:], lb_sb[:, ch * 128:(ch + 1) * 128],
                                r_sb[:], start=False, stop=True)
                            nc.scalar.activation(
                                out=dest[:, t4 * TOK:(t4 + 1) * TOK],
                                in_=ps_q[:], func=COPYF,
                                bias=bias_sb[:, ch:ch + 1], scale=1.0)

                # ---- V -> token-major (+ ones cols) ----
                vaA = vaugp.tile([128, 16 * 66], F16, tag="vaA")
                vaB = vaugp.tile([128, 16 * 66], F16, tag="vaB")
                vaAv = vaA[:].rearrange("p (j c) -> p j c", c=66)
                vaBv = vaB[:].rearrange("p (j c) -> p j c", c=66)
                with nc.named_scope(f"vtr{b}"):
                    nc.sync.dma_start(vaAv[:, :, 64:65],
                                      onesb[:].unsqueeze(-1))
                    nc.sync.dma_start(vaBv[:, :, 64:65],
                                      onesb[:].unsqueeze(-1))
                    for tb in range(16):
                        ps_t = ps.tile([128, 128], F16, tag="mm", bufs=3)
                        nc.tensor.transpose(ps_t[:], vT[:, tb * 128:(tb + 1) * 128],
                                            eye_sb[:])
                        nc.vector.tensor_copy(vaAv[:, tb, 0:64], ps_t[:, 0:64])
                        nc.vector.tensor_copy(vaBv[:, tb, 0:64], ps_t[:, 64:128])

                # ---- attention (both heads interleaved for PE packing) ----
                yt = ytp.tile([128, T], F16, tag="yt")
                with nc.named_scope(f"attn{b}"):
                    for t4 in range(NT4):
                        nblk = 4 * (t4 + 1)
                        q0s, exps = {}, {}
                        psy0 = ps.tile([65, TOK], F32, tag="accy", bufs=2)
                        psy1 = ps.tile([65, TOK], F32, tag="accy", bufs=2)
                        psy = {0: psy0, 1: psy1}

                        def emit_qk(j, h, t4=t4, q0s=q0s, exps=exps):
                            hp = h * 64
                            r = j - 4 * t4
                            q0 = 128 * r if r > 0 else 0
                            q0s[j] = q0
                            ps_s = ps.tile([128, TOK], F32, tag="mm", bufs=3)
                            nc.tensor.matmul(
                                ps_s[:, q0:TOK],
                                kT[hp:hp + 64, j * 128:(j + 1) * 128],
                                qT[hp:hp + 64, t4 * TOK + q0:(t4 + 1) * TOK],
                                start=True, stop=True)
                            e = expp.tile([128, TOK], F16, tag="expS")
                            nc.scalar.activation(
                                out=e[:, q0:TOK], in_=ps_s[:, q0:TOK],
                                func=EXPF, scale=0.125)
                            if r >= 0:
                                nc.vector.tensor_mul(
                                    e[:, q0:q0 + 128], e[:, q0:q0 + 128],
                                    tri_sb[:])
                            exps[(j, h)] = e

                        def emit_pv(j, h, nblk=nblk, q0s=q0s, exps=exps,
                                    psy=psy, vaA=vaA, vaB=vaB):
                            q0 = q0s[j]
                            va = vaA if h == 0 else vaB
                            nc.tensor.matmul(
                                psy[h][:, q0:TOK],
                                va[:, j * 66:j * 66 + 65],
                                exps.pop((j, h))[:, q0:TOK],
                                start=(j == 0), stop=(j == nblk - 1))

                        for h in (0, 1):
                            emit_qk(0, h)
                        if nblk > 1:
                            for h in (0, 1):
                                emit_qk(1, h)
                        for j in range(nblk):
                            if j + 2 < nblk:
                                for h in (0, 1):
                                    emit_qk(j + 2, h)
                            for h in (0, 1):
                                emit_pv(j, h)

                        tsl = slice(t4 * TOK, (t4 + 1) * TOK)
                        for h in (0, 1):
                            zrow = small.tile([65, TOK], F32, tag="zrow")
                            nc.vector.tensor_copy(zrow[64:65, :],
                                                  psy[h][64:65, :])
                            z0 = small.tile([1, TOK], F32, tag="z0")
                            nc.sync.dma_start(z0[:], zrow[64:65, :])
                            recipf = small.tile([1, TOK], F32, tag="recipf")
                            nc.vector.reciprocal_approx_fast(
                                out=recipf[:], in_=z0[:])
                            sb_b = small.tile([64, TOK], F32, tag="sbb")
                            nc.gpsimd.partition_broadcast(sb_b[:], recipf[:])
                            if h == 0:
                                nc.vector.tensor_mul(yt[0:64, tsl],
                                                     psy[0][0:64, :], sb_b[:])
                            else:
                                stage = small.tile([64, TOK], F16, tag="stage")
                                nc.vector.tensor_mul(stage[:], psy[1][0:64, :],
                                                     sb_b[:])
                                nc.sync.dma_start(yt[64:128, tsl], stage[:])

                # ---- output projection (+ LoRA), row-parallel partial ----
                with nc.named_scope(f"proj{b}"):
                    for t4 in range(NT4):
                        gcol = b * T + t4 * TOK
                        tsl = slice(t4 * TOK, (t4 + 1) * TOK)
                        ps_rp = ps.tile([RANK, TOK], F32, tag="accr", bufs=1)
                        nc.tensor.matmul(ps_rp[:], ap_sb[:], yt[:, tsl],
                                         start=True, stop=True)
                        rp_sb = small.tile([RANK, TOK], F16, tag="rp")
                        nc.vector.tensor_copy(rp_sb[:], ps_rp[:])
                        for co in range(NCO):
                            ps_o = ps.tile([128, TOK], F32, tag="mm", bufs=3)
                            nc.tensor.matmul(
                                ps_o[:], wp_sb[:, co * 128:(co + 1) * 128],
                                yt[:, tsl], start=True, stop=False)
                            nc.tensor.matmul(
                                ps_o[:], pb_sb[:, co * 128:(co + 1) * 128],
                                rp_sb[:], start=False, stop=True)
                            po = small.tile([128, TOK], F16, tag="po", bufs=3)
                            nc.any.tensor_copy(po[:], ps_o[:])
                            nc.sync.dma_start(
                                outT[co * 128:(co + 1) * 128, gcol:gcol + TOK],
                                po[:])
    nc.compile()
    return nc


def _prep_inputs(x, W_attn, b_attn, A_attn, B_attn, W_proj, b_proj, A_proj,
                 B_proj):
    xT = np.ascontiguousarray(x.reshape(BT, C).T)
    AqT = np.ascontiguousarray(A_attn.T).reshape(NCIN, 128, RANK)
    ApT_full = A_proj  # [RANK, C]
    tri = np.triu(np.ones((128, 128), np.float32))
    ones64 = np.ones((1, 64), np.float32)
    eye = np.eye(128, dtype=np.float32)
    Bp_s = np.ascontiguousarray((B_proj * SCALING).T)  # [RANK, C]
    in_maps = []
    for c in range(NCORES):
        rows = np.r_[128 * c:128 * c + 128,
                     C + 128 * c:C + 128 * c + 128,
                     2 * C + 128 * c:2 * C + 128 * c + 128]
        W_sl = W_attn[rows]                                  # [384, C]
        WqT = np.ascontiguousarray(W_sl.T).reshape(NCIN, 128, 384)
        b_sl = np.ascontiguousarray(b_attn[rows].reshape(3, 128).T)
        Bq_s = np.ascontiguousarray((B_attn[rows] * SCALING).T)  # [RANK, 384]
        ysl = slice(128 * c, 128 * c + 128)
        WpT = np.ascontiguousarray(W_proj[:, ysl].T)         # [128, C]
        ApT = np.ascontiguousarray(ApT_full[:, ysl].T)       # [128, RANK]
        h = np.float16
        in_maps.append({
            "xT": xT.astype(h), "Wq": WqT.astype(h), "bq": b_sl,
            "Aq": AqT.astype(h), "Bq": Bq_s.astype(h), "Wp": WpT.astype(h),
            "Ap": ApT.astype(h), "Bp": Bp_s.astype(h), "tri": tri.astype(h),
            "ones64": ones64.astype(h), "eye": eye.astype(h),
            "onesb": np.ones((128, 16), h),
        })
    return in_maps


def _install_ntff_shim():
    """Provide antenv.axon_hooks (missing on this image) via ctypes against
    the axon .so, mirroring trn_agent_boot.trn_boot._ntff_profile_via_ctypes."""
    import types
    import ctypes
    import contextlib
    try:
        from antenv.axon_hooks import get_axon_ntff_profile_hook  # noqa: F401
        return
    except ImportError:
        pass
    so_path = "/opt/axon/libaxon_pjrt.so"
    try:
        lib = ctypes.CDLL(so_path)
    except OSError:
        return
    if not hasattr(lib, "axon_start_nrt_profile"):
        return
    lib.axon_start_nrt_profile.argtypes = [ctypes.POINTER(ctypes.c_int64),
                                           ctypes.c_size_t]
    lib.axon_start_nrt_profile.restype = ctypes.c_int64
    lib.axon_stop_nrt_profile.argtypes = [ctypes.c_char_p]
    lib.axon_stop_nrt_profile.restype = ctypes.c_int64

    @contextlib.contextmanager
    def _hook(output_dir, device_ids):
        import jax
        jax.devices()
        if device_ids:
            ids = (ctypes.c_int64 * len(device_ids))(*device_ids)
            rc = lib.axon_start_nrt_profile(ids, len(device_ids))
        else:
            rc = lib.axon_start_nrt_profile(None, 0)
        if rc != 0:
            raise RuntimeError(f"axon_start_nrt_profile rc={rc}")
        try:
            yield
        finally:
            n = lib.axon_stop_nrt_profile(str(output_dir).encode())
            if n < 0:
                raise RuntimeError(f"axon_stop_nrt_profile rc={n}")

    import antenv
    mod = types.ModuleType("antenv.axon_hooks")
    mod.get_axon_ntff_profile_hook = lambda: _hook
    mod.set_axon_ntff_profile_hook = lambda h: None
    sys.modules["antenv.axon_hooks"] = mod
    antenv.axon_hooks = mod


def run(inputs, trace=False, trace_cores=None):
    """Run the kernel. Returns (output, BassKernelResults)."""
    if "nc" not in _cache:
        _cache["nc"] = _build()
    nc = _cache["nc"]
    inputs = {k: np.asarray(v, dtype=np.float32) for k, v in inputs.items()}
    in_maps = _prep_inputs(**inputs)
    if trace:
        _install_ntff_shim()
    res = run_bass_kernel_spmd(nc, in_maps, core_ids=list(range(NCORES)),
                               trace=trace, trace_cores=trace_cores)
    outT = np.zeros((C, BT), np.float32)
    for r in res.results:
        outT += r["outT"].astype(np.float32)
    out = outT.T + inputs["b_proj"][None, :]
    return out.astype(np.float32).reshape(B, T, C), res


def kernel(**inputs):
    out, _ = run(inputs, trace=False)
    return out


# revision 16
# speedup vs baseline: 1.6091x; 1.0771x over previous
"""Trainium2 Bass kernel for causal multi-head attention with LoRA (QKV + proj).

Problem (hardcoded): B=4, T=2048, C=1024, NH=16, HD=64, RANK=56, alpha=8.

Sharding: tensor-parallel across heads — each of the 8 cores owns 2 heads
(128 qkv dims per projection) and processes all 4 batches. The output
projection is row-parallel (each core contracts over its own 128 y dims);
partial outputs are summed on the host.

All matmuls run in float32r (TF32-like, ~1e-4 relative rounding, full PE
rate for moving dims >= 256). Layout is transposed throughout: activations
live as [feature(partition), token(free)], which makes the QKV projection,
QK^T, PV and output projection all natural matmuls. The only on-chip
transposes are V (needed token-major for PV): 16 PE transposes per batch.

Softmax: scoresT [tk, tq] -> exp on ACT (scale=1/8 folded in); causal
masking via block-sliced matmul ranges + one [128,128] triangular mask
multiply per diagonal block; row sums via an appended ones column in the
PV stationary operand; normalization via reciprocal + K=1 broadcast matmul.
"""
import sys
import numpy as np

if "/opt/trn_rl_repo" not in sys.path:
    sys.path.insert(0, "/opt/trn_rl_repo")

import concourse.bass as bass  # noqa: E402
from concourse import bacc  # noqa: E402
import concourse.mybir as mybir  # noqa: E402
import concourse.tile as tile  # noqa: E402
from concourse.bass_utils import run_bass_kernel_spmd  # noqa: E402

B, T, C = 4, 2048, 1024
NH, HD, RANK = 16, 64, 56
SCALING = 8.0 / 56.0
NCORES = 8
BT = B * T            # 8192
TOK = 512             # token chunk (matmul moving dim)
NT4 = T // TOK        # 4 token chunks per batch
NCIN = C // 128       # 8 input-feature chunks
NCO = C // 128        # 8 output-feature chunks (proj)
F32 = mybir.dt.float32
F32R = mybir.dt.float32r
F16 = mybir.dt.float16
EXPF = mybir.ActivationFunctionType.Exp
COPYF = mybir.ActivationFunctionType.Identity  # Copy rejects AP bias

_cache = {}


def _build():
    nc = bacc.Bacc("TRN2", target_bir_lowering=False, debug=False,
                   num_devices=NCORES)
    xT = nc.dram_tensor("xT", [C, BT], F16, kind="ExternalInput")
    Wq = nc.dram_tensor("Wq", [NCIN, 128, 384], F16, kind="ExternalInput")
    bq = nc.dram_tensor("bq", [128, 3], F32, kind="ExternalInput")
    Aq = nc.dram_tensor("Aq", [NCIN, 128, RANK], F16, kind="ExternalInput")
    Bq = nc.dram_tensor("Bq", [128, 384], F16, kind="ExternalInput")
    Wp = nc.dram_tensor("Wp", [128, C], F16, kind="ExternalInput")
    Ap = nc.dram_tensor("Ap", [128, RANK], F16, kind="ExternalInput")
    Bp = nc.dram_tensor("Bp", [128, C], F16, kind="ExternalInput")
    tri = nc.dram_tensor("tri", [128, 128], F16, kind="ExternalInput")
    ones64 = nc.dram_tensor("ones64", [1, 64], F16, kind="ExternalInput")
    eye = nc.dram_tensor("eye", [128, 128], F16, kind="ExternalInput")
    onesb = nc.dram_tensor("onesb", [128, 16], F16, kind="ExternalInput")
    zeros96 = nc.dram_tensor("zeros96", [128, TOK], F16, kind="ExternalInput")
    outT = nc.dram_tensor("outT", [C, BT], F16, kind="ExternalOutput")

    with tile.TileContext(nc) as tc:
        with (
            tc.tile_pool(name="consts", bufs=1) as consts,
            tc.tile_pool(name="qkv", bufs=2) as qkvp,
            tc.tile_pool(name="vaugp", bufs=2) as vaugp,
            tc.tile_pool(name="ytp", bufs=2) as ytp,
            tc.tile_pool(name="xtp", bufs=16) as xtp,
            tc.tile_pool(name="expp", bufs=8) as expp,
            tc.tile_pool(name="small", bufs=2) as small,
            tc.tile_pool(name="ps", bufs=1, space="PSUM") as ps,
        ):
            wq_sb = consts.tile([128, NCIN, 384], F16)
            nc.sync.dma_start(wq_sb[:], Wq[:].rearrange("c p f -> p c f"))
            aq_sb = consts.tile([128, NCIN, RANK], F16)
            nc.sync.dma_start(aq_sb[:], Aq[:].rearrange("c p f -> p c f"))
            lb_sb = consts.tile([128, 384], F16)
            nc.sync.dma_start(lb_sb[:], Bq[:])
            bias_sb = consts.tile([128, 3], F32)
            nc.sync.dma_start(bias_sb[:], bq[:])
            wp_sb = consts.tile([128, C], F16)
            nc.sync.dma_start(wp_sb[:], Wp[:])
            ap_sb = consts.tile([128, RANK], F16)
            nc.sync.dma_start(ap_sb[:], Ap[:])
            pb_sb = consts.tile([128, C], F16)
            nc.sync.dma_start(pb_sb[:], Bp[:])
            tri_sb = consts.tile([128, 128], F16)
            nc.sync.dma_start(tri_sb[:], tri[:])
            ones_sb = consts.tile([1, 64], F16)
            nc.sync.dma_start(ones_sb[:], ones64[:])
            eye_sb = consts.tile([128, 128], F16)
            nc.sync.dma_start(eye_sb[:], eye[:])
            zeros_sb = consts.tile([128, TOK], F16)
            nc.sync.dma_start(zeros_sb[:], zeros96[:])

            for b in range(B):
                # ---- QKV projection (+ LoRA) for batch b ----
                qT = qkvp.tile([128, T], F16, tag="qT")
                kT = qkvp.tile([128, T], F16, tag="kT")
                vT = qkvp.tile([128, T], F16, tag="vT")
                with nc.named_scope(f"qkv{b}"):
                    for t4 in range(NT4):
                        gcol = b * T + t4 * TOK
                        xts = []
                        for cc in range(NCIN):
                            xt = xtp.tile([128, TOK], F16, tag="xt")
                            nc.sync.dma_start(
                                xt[:],
                                xT[cc * 128:(cc + 1) * 128, gcol:gcol + TOK])
                            xts.append(xt)
                        ps_r = ps.tile([RANK, TOK], F32, tag="accr", bufs=1)
                        for cc in range(NCIN):
                            nc.tensor.matmul(ps_r[:], aq_sb[:, cc, :], xts[cc][:],
                                             start=(cc == 0), stop=(cc == NCIN - 1))
                        r_sb = small.tile([128, TOK], F16, tag="r")
                        nc.vector.tensor_copy(r_sb[32:64, :], zeros_sb[32:64, :])
                        nc.vector.tensor_copy(r_sb[64:128, :], zeros_sb[64:128, :])
                        nc.vector.tensor_copy(r_sb[0:RANK, :], ps_r[:])
                        for ch, dest in enumerate((qT, kT, vT)):
                            ps_q = ps.tile([128, TOK], F32, tag="acc", bufs=2)
                            for cc in range(NCIN):
                                nc.tensor.matmul(
                                    ps_q[:],
                                    wq_sb[:, cc, ch * 128:(ch + 1) * 128],
                                    xts[cc][:], start=(cc == 0), stop=False)
                            nc.tensor.matmul(
                                ps_q[:], lb_sb[:, ch * 128:(ch + 1) * 128],
                                r_sb[:], start=False, stop=True)
                            nc.scalar.activation(
                                out=dest[:, t4 * TOK:(t4 + 1) * TOK],
                                in_=ps_q[:], func=COPYF,
                                bias=bias_sb[:, ch:ch + 1], scale=1.0)

                # ---- V -> token-major (+ ones cols) ----
                vaA = vaugp.tile([128, 16 * 66], F16, tag="vaA")
                vaB = vaugp.tile([128, 16 * 66], F16, tag="vaB")
                vaAv = vaA[:].rearrange("p (j c) -> p j c", c=66)
                vaBv = vaB[:].rearrange("p (j c) -> p j c", c=66)
                with nc.named_scope(f"vtr{b}"):
                    for col in (64, 65):
                        nc.sync.dma_start(vaAv[:, :, col:col + 1],
                                          onesb[:].unsqueeze(-1))
                        nc.sync.dma_start(vaBv[:, :, col:col + 1],
                                          onesb[:].unsqueeze(-1))
                    for tb in range(16):
                        ps_t = ps.tile([128, 128], F16, tag="mm", bufs=3)
                        nc.tensor.transpose(ps_t[:], vT[:, tb * 128:(tb + 1) * 128],
                                            eye_sb[:])
                        nc.vector.tensor_copy(vaAv[:, tb, 0:64], ps_t[:, 0:64])
                        nc.vector.tensor_copy(vaBv[:, tb, 0:64], ps_t[:, 64:128])

                # ---- attention (both heads interleaved for PE packing) ----
                yt = ytp.tile([128, T], F16, tag="yt")
                with nc.named_scope(f"attn{b}"):
                    for t4 in range(NT4):
                        nblk = 4 * (t4 + 1)
                        q0s, exps = {}, {}
                        psy0 = ps.tile([66, TOK], F32, tag="accy", bufs=2)
                        psy1 = ps.tile([66, TOK], F32, tag="accy", bufs=2)
                        psy = {0: psy0, 1: psy1}

                        def emit_qk(j, h, t4=t4, q0s=q0s, exps=exps):
                            hp = h * 64
                            r = j - 4 * t4
                            q0 = 128 * r if r > 0 else 0
                            q0s[j] = q0
                            ps_s = ps.tile([128, TOK], F32, tag="mm", bufs=3)
                            nc.tensor.matmul(
                                ps_s[:, q0:TOK],
                                kT[hp:hp + 64, j * 128:(j + 1) * 128],
                                qT[hp:hp + 64, t4 * TOK + q0:(t4 + 1) * TOK],
                                start=True, stop=True)
                            e = expp.tile([128, TOK], F16, tag="expS")
                            nc.scalar.activation(
                                out=e[:, q0:TOK], in_=ps_s[:, q0:TOK],
                                func=EXPF, scale=0.125)
                            if r >= 0:
                                nc.vector.tensor_mul(
                                    e[:, q0:q0 + 128], e[:, q0:q0 + 128],
                                    tri_sb[:])
                            exps[(j, h)] = e

                        def emit_pv(j, h, nblk=nblk, q0s=q0s, exps=exps,
                                    psy=psy, vaA=vaA, vaB=vaB):
                            q0 = q0s[j]
                            va = vaA if h == 0 else vaB
                            nc.tensor.matmul(
                                psy[h][:, q0:TOK],
                                va[:, j * 66:j * 66 + 66],
                                exps.pop((j, h))[:, q0:TOK],
                                start=(j == 0), stop=(j == nblk - 1))

                        for h in (0, 1):
                            emit_qk(0, h)
                        if nblk > 1:
                            for h in (0, 1):
                                emit_qk(1, h)
                        for j in range(nblk):
                            if j + 2 < nblk:
                                for h in (0, 1):
                                    emit_qk(j + 2, h)
                            for h in (0, 1):
                                emit_pv(j, h)

                        tsl = slice(t4 * TOK, (t4 + 1) * TOK)
                        for h in (0, 1):
                            zrow = small.tile([65, TOK], F32, tag="zrow")
                            nc.vector.tensor_copy(zrow[64:65, :],
                                                  psy[h][64:65, :])
                            z0 = small.tile([1, TOK], F32, tag="z0")
                            nc.sync.dma_start(z0[:], zrow[64:65, :])
                            recipf = small.tile([1, TOK], F32, tag="recipf")
                            nc.vector.reciprocal_approx_fast(
                                out=recipf[:], in_=z0[:])
                            sb_b = small.tile([64, TOK], F32, tag="sbb")
                            nc.gpsimd.partition_broadcast(sb_b[:], recipf[:])
                            if h == 0:
                                nc.vector.tensor_mul(yt[0:64, tsl],
                                                     psy[0][0:64, :], sb_b[:])
                            else:
                                stage = small.tile([64, TOK], F16, tag="stage")
                                nc.vector.tensor_mul(stage[:], psy[1][0:64, :],
                                                     sb_b[:])
                                nc.sync.dma_start(yt[64:128, tsl], stage[:])

                # ---- output projection (+ LoRA), row-parallel partial ----
                with nc.named_scope(f"proj{b}"):
                    for t4 in range(NT4):
                        gcol = b * T + t4 * TOK
                        tsl = slice(t4 * TOK, (t4 + 1) * TOK)
                        ps_rp = ps.tile([RANK, TOK], F32, tag="accr", bufs=1)
                        nc.tensor.matmul(ps_rp[:], ap_sb[:], yt[:, tsl],
                                         start=True, stop=True)
                        rp_sb = small.tile([128, TOK], F16, tag="rp")
                        nc.vector.tensor_copy(rp_sb[32:64, :], zeros_sb[32:64, :])
                        nc.vector.tensor_copy(rp_sb[64:128, :], zeros_sb[64:128, :])
                        nc.vector.tensor_copy(rp_sb[0:RANK, :], ps_rp[:])
                        for co in range(NCO):
                            ps_o = ps.tile([128, TOK], F32, tag="mm", bufs=3)
                            nc.tensor.matmul(
                                ps_o[:], wp_sb[:, co * 128:(co + 1) * 128],
                                yt[:, tsl], start=True, stop=False)
                            nc.tensor.matmul(
                                ps_o[:], pb_sb[:, co * 128:(co + 1) * 128],
                                rp_sb[:], start=False, stop=True)
                            po = small.tile([128, TOK], F16, tag="po", bufs=3)
                            nc.any.tensor_copy(po[:], ps_o[:])
                            nc.sync.dma_start(
                                outT[co * 128:(co + 1) * 128, gcol:gcol + TOK],
                                po[:])
    nc.compile()
    return nc


def _prep_inputs(x, W_attn, b_attn, A_attn, B_attn, W_proj, b_proj, A_proj,
                 B_proj):
    xT = np.ascontiguousarray(x.reshape(BT, C).T)
    AqT = np.ascontiguousarray(A_attn.T).reshape(NCIN, 128, RANK)
    ApT_full = A_proj  # [RANK, C]
    tri = np.triu(np.ones((128, 128), np.float32))
    ones64 = np.ones((1, 64), np.float32)
    eye = np.eye(128, dtype=np.float32)
    Bp_s = np.zeros((128, C), np.float32)
    Bp_s[:RANK] = (B_proj * SCALING).T
    in_maps = []
    for c in range(NCORES):
        rows = np.r_[128 * c:128 * c + 128,
                     C + 128 * c:C + 128 * c + 128,
                     2 * C + 128 * c:2 * C + 128 * c + 128]
        W_sl = W_attn[rows]                                  # [384, C]
        WqT = np.ascontiguousarray(W_sl.T).reshape(NCIN, 128, 384)
        b_sl = np.ascontiguousarray(b_attn[rows].reshape(3, 128).T)
        Bq_s = np.zeros((128, 384), np.float32)
        Bq_s[:RANK] = (B_attn[rows] * SCALING).T
        ysl = slice(128 * c, 128 * c + 128)
        WpT = np.ascontiguousarray(W_proj[:, ysl].T)         # [128, C]
        ApT = np.ascontiguousarray(ApT_full[:, ysl].T)       # [128, RANK]
        h = np.float16
        in_maps.append({
            "xT": xT.astype(h), "Wq": WqT.astype(h), "bq": b_sl,
            "Aq": AqT.astype(h), "Bq": Bq_s.astype(h), "Wp": WpT.astype(h),
            "Ap": ApT.astype(h), "Bp": Bp_s.astype(h), "tri": tri.astype(h),
            "ones64": ones64.astype(h), "eye": eye.astype(h),
            "onesb": np.ones((128, 16), h),
            "zeros96": np.zeros((128, TOK), h),
        })
    return in_maps


def _install_ntff_shim():
    """Provide antenv.axon_hooks (missing on this image) via ctypes against
    the axon .so, mirroring trn_agent_boot.trn_boot._ntff_profile_via_ctypes."""
    import types
    import ctypes
    import contextlib
    try:
        from antenv.axon_hooks import get_axon_ntff_profile_hook  # noqa: F401
        return
    except ImportError:
        pass
    so_path = "/opt/axon/libaxon_pjrt.so"
    try:
        lib = ctypes.CDLL(so_path)
    except OSError:
        return
    if not hasattr(lib, "axon_start_nrt_profile"):
        return
    lib.axon_start_nrt_profile.argtypes = [ctypes.POINTER(ctypes.c_int64),
                                           ctypes.c_size_t]
    lib.axon_start_nrt_profile.restype = ctypes.c_int64
    lib.axon_stop_nrt_profile.argtypes = [ctypes.c_char_p]
    lib.axon_stop_nrt_profile.restype = ctypes.c_int64

    @contextlib.contextmanager
    def _hook(output_dir, device_ids):
        import jax
        jax.devices()
        if device_ids:
            ids = (ctypes.c_int64 * len(device_ids))(*device_ids)
            rc = lib.axon_start_nrt_profile(ids, len(device_ids))
        else:
            rc = lib.axon_start_nrt_profile(None, 0)
        if rc != 0:
            raise RuntimeError(f"axon_start_nrt_profile rc={rc}")
        try:
            yield
        finally:
            n = lib.axon_stop_nrt_profile(str(output_dir).encode())
            if n < 0:
                raise RuntimeError(f"axon_stop_nrt_profile rc={n}")

    import antenv
    mod = types.ModuleType("antenv.axon_hooks")
    mod.get_axon_ntff_profile_hook = lambda: _hook
    mod.set_axon_ntff_profile_hook = lambda h: None
    sys.modules["antenv.axon_hooks"] = mod
    antenv.axon_hooks = mod


def run(inputs, trace=False, trace_cores=None):
    """Run the kernel. Returns (output, BassKernelResults)."""
    if "nc" not in _cache:
        _cache["nc"] = _build()
    nc = _cache["nc"]
    inputs = {k: np.asarray(v, dtype=np.float32) for k, v in inputs.items()}
    in_maps = _prep_inputs(**inputs)
    if trace:
        _install_ntff_shim()
    res = run_bass_kernel_spmd(nc, in_maps, core_ids=list(range(NCORES)),
                               trace=trace, trace_cores=trace_cores)
    outT = np.zeros((C, BT), np.float32)
    for r in res.results:
        outT += r["outT"].astype(np.float32)
    out = outT.T + inputs["b_proj"][None, :]
    return out.astype(np.float32).reshape(B, T, C), res


def kernel(**inputs):
    out, _ = run(inputs, trace=False)
    return out
